# revision 62
# baseline (speedup 1.0000x reference)
"""Trainium2 Bass kernel for a fake-quantized MLP (qlinear -> gelu -> qlinear).

Reference semantics (B,S,C,H = 32,1024,1024,4096):
    x2d = x.reshape(-1, C)
    h   = round(x2d/sx) @ round(w1/sw1).T * (sx*sw1) + b1 ;  s = max(amax,eps)/127
    g   = gelu(h, exact erf)
    y   = round(g/sh) @ round(w2/sw2).T * (sh*sw2) + b2

Strategy: data-parallel over rows across 8 cores.  Quantized ints fit
exactly in bf16, so matmuls run at full bf16 rate with exact fp32 PSUM
accumulation.  Weight scales need no collective (every core scans the full
weights); the two activation scales (x, gelu output) use tiny
AllReduce(max) collectives.

v2 layout changes vs the first working version:
  - all quantized-tile transposes go through the DMA xbar (SBUF->SBUF for
    x/w1 block transposes, DRAM->SBUF bulk transpose-load for w2) instead
    of PE identity matmuls + ACT copies: keeps PE for real matmuls and
    unclogs the ACT engine that previously gated w1 prep.
  - gelu output h is staged to DRAM in fp16 (not fp32): halves the
    mid-kernel HBM traffic; fp16 rounding of g only perturbs the final
    int8 requant on ~0.1% boundary cases (noise ~1e-3 of output max).
  - w2 amax scan + quant + store of quantized-natural w2q (bf16, DRAM)
    are trickled through phase A; at the phase A->B transition w2qT is
    materialized with 8 bulk DMA transpose-loads that hide under the
    h-scale AllReduce.
  - x amax scan is emitted first so its AllReduce launches as early as
    possible; w1 scan + quant pipeline fills the AllReduce latency.
"""

import sys

import numpy as np

try:
    import concourse.bass as bass
except ImportError:  # pragma: no cover
    sys.path.insert(0, "/opt/trn_rl_repo")
    import concourse.bass as bass

import concourse.mybir as mybir
from contextlib import ExitStack
import concourse.tile as tile
from concourse.bass_utils import run_bass_kernel_spmd

from concourse.bass import _add_dep_helper as _add_dep

F32 = mybir.dt.float32
F16 = mybir.dt.float16
BF16 = mybir.dt.bfloat16
AF = mybir.ActivationFunctionType
ALU = mybir.AluOpType

QP = 127.0
EPS = 1e-6
MAGIC = 12582912.0  # 1.5 * 2**23: fp32 round-to-nearest-even integer trick

# full problem shapes
B, S, C, H = 32, 1024, 1024, 4096
N_CORES = 8


def _split_matmul_waits(nc):
    """This toolchain's walrus codegen allows only ONE sync-wait slot per
    lowered instruction (Matmult waits all land on its LDWEIGHTS since
    --enable-ldw-opt=false; queue DMAs use a single-slot DIRECT2D struct).
    Peel extra waits onto same-engine NoOps inserted just before, except for
    framework-generated drain/barrier instructions which support many."""
    n_split = 0
    for f in nc.m.functions:
        for bb in f.blocks:
            insts = bb.instructions
            out = []
            changed = False
            for inst in insts:
                si = getattr(inst, "sync_info", None)
                if si is not None and si.on_wait and len(si.on_wait) > 1:
                    waits = list(si.on_wait)
                    for k, w in enumerate(waits[:-1]):
                        nop = mybir.InstNoOp(
                            name=f"{inst.name}-wsplit{k}", ins=[], outs=[]
                        )
                        nop.engine = inst.engine
                        nop.sync_info = mybir.SyncInfo(
                            on_wait=[w], on_update=[]
                        )
                        out.append(nop)
                    inst.sync_info = mybir.SyncInfo(
                        on_wait=[waits[-1]], on_update=list(si.on_update or [])
                    )
                    n_split += 1
                    changed = True
                out.append(inst)
            if changed:
                bb.instructions = out
    return n_split


def _dedup_ldweights(nc):
    """Tile legalization emits explicit Ldweights+Matmult pairs, and walrus
    runs with --enable-ldw-opt=false, so every matmul re-streams its
    stationary operand (128 extra PE cycles on a 512-cycle matmul).  Drop an
    Ldweights whose weights AP is identical to the previous one on the PE
    stream (the PE array still holds that stationary); keep its semaphore
    effects on a NoOp."""
    n = 0
    for f in nc.m.functions:
        for bb in f.blocks:
            insts = bb.instructions
            out = []
            last_key = None
            changed = False
            for inst in insts:
                if isinstance(inst, mybir.InstLdweights):
                    key = str(inst.ins[0])
                    if key == last_key:
                        si = getattr(inst, "sync_info", None)
                        if si is not None and (si.on_wait or si.on_update):
                            nop = mybir.InstNoOp(
                                name=inst.name + "-lw", ins=[], outs=[]
                            )
                            nop.engine = inst.engine
                            nop.sync_info = si
                            out.append(nop)
                        n += 1
                        changed = True
                        continue
                    last_key = key
                elif isinstance(inst, mybir.InstMatmult):
                    if inst.is_transpose or getattr(inst, "ldweights", None):
                        last_key = None
                out.append(inst)
            if changed:
                bb.instructions = out
    return n


def build_nc(rows=4096, c=C, h=H, n_cores=N_CORES, gelu="Gelu", split_waits=True):
    """Build the per-core SPMD Bass program.

    rows: rows of x2d handled by each core.
    gelu: "Gelu" (HW ACT table), "Erf" (x*(0.5*erf(x/sqrt2)+0.5)),
          "Identity" (for simulator runs; CoreSim lacks Gelu/Erf).
    """
    assert rows % 512 == 0 and c % 512 == 0 and h % 512 == 0
    nc = bass.Bass()

    x_in = nc.dram_tensor("x", [rows, c], F32, kind="ExternalInput")
    w1_in = nc.dram_tensor("w1", [h, c], F32, kind="ExternalInput")
    b1_in = nc.dram_tensor("b1", [h], F32, kind="ExternalInput")
    w2_in = nc.dram_tensor("w2", [c, h], F32, kind="ExternalInput")
    b2_in = nc.dram_tensor("b2", [c], F32, kind="ExternalInput")
    # per-core weight shards (sliced in run()) so the amax scans are
    # 1/n_cores the traffic; the partial maxes AllReduce to the global max
    w1s_in = nc.dram_tensor("w1s", [h // n_cores, c], F32, kind="ExternalInput")
    w2s_in = nc.dram_tensor("w2s", [c // n_cores, h], F32, kind="ExternalInput")
    y_out = nc.dram_tensor("y", [rows, c], F32, kind="ExternalOutput")

    ct = c // 128   # c in 128-blocks
    ht = h // 128   # h in 128-blocks
    groups = [list(range(n_cores))]

    with tile.TileContext(nc) as tc, ExitStack() as top:
        consts = top.enter_context(tc.tile_pool(name="consts", bufs=1))
        scal = top.enter_context(tc.tile_pool(name="scal", bufs=1))
        dram = top.enter_context(tc.tile_pool(name="dram", bufs=1, space="DRAM"))

        ident_f = consts.tile([128, 128], F32)
        from concourse import masks
        masks.make_identity(nc, ident_f[:])
        ident = consts.tile([128, 128], BF16)
        masks.make_identity(nc, ident[:])

        # b1 as (128, ht): b1_sb[p, jb] = b1[jb*128 + p]
        b1_sb = consts.tile([128, ht], F32)
        nc.sync.dma_start(
            out=b1_sb[:], in_=b1_in.ap().rearrange("(a b) -> b a", b=128)
        )

        magic_b = consts.tile([128, 1], F32)
        nc.vector.memset(magic_b[:], MAGIC)

        # b2 broadcast early: its 128x4KB broadcast packets should ride the
        # idle prologue ring, not the busy phase-B transition
        b2_b = consts.tile([128, c], F32)
        nc.sync.dma_start(
            out=b2_b[:],
            in_=b2_in.ap().rearrange("(o a) -> o a", o=1).to_broadcast((128, c)),
        )

        # h scratch in DRAM, transposed + fp16: (h, rows)
        h_dram = dram.tile([h, rows], F16)
        # quantized-natural staging (bf16) for the bulk DMA transpose-loads.
        # Per-block SBUF->SBUF xbar transposes degrade into 256B packets on a
        # single HWDGE ring (measured ~25x DMA throughput collapse); bulk
        # DRAM->SBUF transpose-loads with contiguous >=1KB source rows run at
        # ~280GB/s.  x and w2 bounce through DRAM; w1 (prologue critical
        # path, PE idle anyway) is transposed on the PE instead to keep
        # 17MB off the sync ring.
        xq_dram = dram.tile([rows, c], BF16, tag="xqd")
        w2q_dram = dram.tile([c, h], BF16, tag="w2qd")
        # collective bounce buffers (DRAM, non-IO).  arx carries [xmax, w1max]
        # in one 2-element AllReduce; arw carries the sharded w2max.
        arx_in = dram.tile([2, 1], F32, tag="arxi")
        arx_out = dram.tile([2, 1], F32, tag="arxo")
        arw_in = dram.tile([1, 1], F32, tag="arwi")
        arw_out = dram.tile([1, 1], F32, tag="arwo")
        arh_in = dram.tile([1, 1], F32, tag="arhi")
        arh_out = dram.tile([1, 1], F32, tag="arho")

        def _preduce(acc, psR, tag):
            """[128,1] partition-max -> [1,1] via PE transpose + DVE reduce."""
            pt = psR.tile([1, 128], F32, tag=tag)
            nc.tensor.matmul(
                pt[:], lhsT=acc[:], rhs=ident_f[:], start=True, stop=True
            )
            out = scal.tile([1, 1], F32, name=tag + "_r")
            nc.vector.tensor_reduce(
                out=out[:], in_=pt[:], axis=mybir.AxisListType.X, op=ALU.max
            )
            return out

        def _derive(bcast_src_dram, name):
            b = scal.tile([128, 1], F32, name=name + "_b")
            nc.sync.dma_start(out=b[:], in_=bcast_src_dram.to_broadcast((128, 1)))
            s = scal.tile([128, 1], F32, name="s_" + name)
            nc.vector.tensor_scalar(
                out=s[:], in0=b[:], scalar1=EPS, scalar2=float(1.0 / QP),
                op0=ALU.max, op1=ALU.mult,
            )
            inv = scal.tile([128, 1], F32, name="inv_" + name)
            nc.vector.reciprocal(out=inv[:], in_=s[:])
            return s, inv

        # ---------- phase 0: x amax scan FIRST (its AllReduce is the long
        # pole: ~50us mesh latency).  512-row tiles, DMA-bound. ----------
        xmax = scal.tile([128, 1], F32)
        nc.vector.memset(xmax[:], 0.0)
        wmax1 = scal.tile([128, 1], F32)
        nc.vector.memset(wmax1[:], 0.0)
        wmax2 = scal.tile([128, 1], F32)
        nc.vector.memset(wmax2[:], 0.0)

        with tc.tile_pool(name="scan", bufs=4) as scanp, tc.tile_pool(
            name="scant", bufs=4
        ) as scant, tc.tile_pool(name="psR", bufs=2, space="PSUM") as psR0:
            for mb in range(rows // 512):
                t = scanp.tile([128, c * 4], F32, tag="sc")
                nc.sync.dma_start(
                    out=t[:].rearrange("b (a c) -> b a c", a=4),
                    in_=x_in[mb * 512 : (mb + 1) * 512, :].rearrange(
                        "(a b) c -> b a c", b=128
                    ),
                )
                r = scant.tile([128, 1], F32, tag="scr")
                nc.vector.tensor_reduce(
                    out=r[:], in_=t[:], axis=mybir.AxisListType.X, op=ALU.max,
                    apply_absolute_value=True,
                )
                nc.vector.tensor_tensor(
                    out=xmax[:], in0=xmax[:], in1=r[:], op=ALU.max
                )
            xmax_r = _preduce(xmax, psR0, "xm")
            nc.gpsimd.dma_start(out=arx_in[0:1, :], in_=xmax_r[:])

            # sharded w1 amax scan: this core's 1/n_cores slice only; the
            # partial max rides the same AllReduce as xmax (2 elements).
            # MUST stay on the sync HWDGE ring: the scalar ring concurrently
            # (in a NEFF that also issues xbar DMA transposes) crashes the
            # device; SWDGE/gpsimd takes ~1ms to software-generate the
            # descriptors for these strided tiles.
            w1s_rows = h // n_cores
            for rb in range((w1s_rows + 511) // 512):
                nrow = min(512, w1s_rows - rb * 512)
                nb = nrow // 128
                t = scanp.tile([128, c * nb], F32, tag="sc")
                nc.sync.dma_start(
                    out=t[:, : c * nb].rearrange("b (a c) -> b a c", a=nb),
                    in_=w1s_in[rb * 512 : rb * 512 + nrow, :].rearrange(
                        "(a b) c -> b a c", b=128
                    ),
                )
                r = scant.tile([128, 1], F32, tag="scr")
                nc.vector.tensor_reduce(
                    out=r[:], in_=t[:, : c * nb], axis=mybir.AxisListType.X,
                    op=ALU.max, apply_absolute_value=True,
                )
                nc.vector.tensor_tensor(
                    out=wmax1[:], in0=wmax1[:], in1=r[:], op=ALU.max
                )
            w1max_r = _preduce(wmax1, psR0, "w1m")
            nc.gpsimd.dma_start(out=arx_in[1:2, :], in_=w1max_r[:])
            nc.gpsimd.collective_compute(
                "AllReduce", ALU.max, replica_groups=groups,
                ins=[arx_in.opt()], outs=[arx_out.opt()],
            )

        sw1, inv_sw1 = _derive(arx_out[1:2, :], "w1")
        sx, inv_sx = _derive(arx_out[0:1, :], "x")
        sxw1 = scal.tile([128, 1], F32)
        nc.vector.tensor_tensor(out=sxw1[:], in0=sx[:], in1=sw1[:], op=ALU.mult)

        hmax = scal.tile([128, 1], F32)
        nc.vector.memset(hmax[:], 0.0)

        # ---------- w1 quant -> w1q_dram -> bulk transpose-loads ----------
        # w1qT_all layout: [128(p=c%128), jg(h/512), cb(ct), 512(j within jg)]
        # so matmul lhsT for (cb, jb) = [:, jb//4, cb, (jb%4)*128:+128].
        # Pipelined per 512-row group; emission order on the single sync ring
        # is chosen so the operands of the first matmuls land first:
        # x chunk-0 f32 loads, w1 jg=0, x chunk-0 quant+transpose, w1 jg>0.
        n_jg1 = h // 512
        w1_stack = ExitStack()
        w1qT_pool = w1_stack.enter_context(
            tc.tile_pool(name="w1qT", bufs=1, side="right")
        )
        w1qT_all = w1qT_pool.tile([128, n_jg1, ct, 512], BF16, tag="w1qT")

        def w1qT_ap(cb, jb):
            return w1qT_all[:, jb // 4, cb, (jb % 4) * 128 : (jb % 4 + 1) * 128]

        # ---------- phase A: h.T = gelu(w1q @ xq.T * (sx*sw1) + b1) -------
        CH = min(1024, rows // 2)
        n_ms = CH // 512
        n_chunks_a = rows // CH
        # w2 plan: sharded amax scan + AllReduce launched during chunk 0;
        # quant+store pieces ([128, h/2]) trickled through the last two
        # chunks (sw2 only needs to be ready by then).
        assert n_chunks_a >= 2
        hw2 = min(1024, h)
        n_w2_pieces = ct * (h // hw2)
        if n_chunks_a >= 4:
            w2_quant_chunks = {2: n_w2_pieces // 2,
                               3: n_w2_pieces - n_w2_pieces // 2}
        else:
            w2_quant_chunks = {1: n_w2_pieces}
        sw2_state = {}

        # Pool lifetime plan (SBUF pools are a LIFO stack; PSUM has its own):
        #   pa0 (gs, gr, xqt)        — open until the very end
        #   inner1 (xa, xq)          — closes after chunk-(n-2) emission
        #   w1pools (w1f, w1q)       — closes after the prologue
        #   w2pools (w2 scan/quant)  — closes after the last w2 piece
        #   w2qTp                    — opens at chunk-(n-1): its tiles land in
        #                              the space w2pools/inner1 just freed, so
        #                              the w2qT transpose-loads ride the ring
        #                              DURING the last chunk instead of
        #                              serializing into the transition.
        pa0 = ExitStack()
        gs = pa0.enter_context(tc.tile_pool(name="gs", bufs=6))
        gr = pa0.enter_context(tc.tile_pool(name="gr", bufs=8))
        xqtp = pa0.enter_context(tc.tile_pool(name="xqt", bufs=2))
        inner1 = ExitStack()
        xa = inner1.enter_context(tc.tile_pool(name="xa", bufs=4))
        xqp = inner1.enter_context(tc.tile_pool(name="xq", bufs=2))
        w1pools = ExitStack()
        w1f = w1pools.enter_context(tc.tile_pool(name="w1f", bufs=6))
        w1qp = w1pools.enter_context(tc.tile_pool(name="w1q", bufs=6))
        psT = w1pools.enter_context(tc.tile_pool(name="psT", bufs=4, space="PSUM"))

        def w1_strip_pipeline(rb):
            """load strip -> quantize -> PE-transpose 128x128 blocks into
            w1qT_all (batched 4 blocks per PSUM bank, one DVE copy each)."""
            wt = w1f.tile([128, c], F32, tag="w1f")
            nc.sync.dma_start(
                out=wt[:], in_=w1_in[rb * 128 : (rb + 1) * 128, :]
            )
            nc.scalar.activation(
                out=wt[:], in_=wt[:], func=AF.Identity, bias=magic_b[:],
                scale=inv_sw1[:],
            )
            wq = w1qp.tile([128, c], BF16, tag="w1q")
            nc.vector.tensor_scalar_add(out=wq[:], in0=wt[:], scalar1=-MAGIC)
            jg, jsub = rb // 4, (rb % 4) * 128
            for cq in range(ct // 4):
                pt = psT.tile([128, 512], F32, tag="psT")
                for k in range(4):
                    cb = cq * 4 + k
                    nc.tensor.matmul(
                        pt[:, k * 128 : (k + 1) * 128],
                        lhsT=wq[:, cb * 128 : (cb + 1) * 128],
                        rhs=ident[:], start=True, stop=True,
                    )
                nc.vector.tensor_copy(
                    out=w1qT_all[:, jg, cq * 4 : (cq + 1) * 4,
                                 jsub : jsub + 128],
                    in_=pt[:].rearrange("p (a b) -> p a b", a=4),
                )

        def x_quant_chunk(mc, pe_transpose=False):
            xqT = xqtp.tile([128, ct, CH], BF16, tag="xqT", name=f"xqT{mc}")
            for t8 in range(CH // 128):
                m0 = mc * CH + t8 * 128
                xt = xa.tile([128, c], F32, tag="xa")
                nc.sync.dma_start(out=xt[:], in_=x_in[m0 : m0 + 128, :])
                nc.scalar.activation(
                    out=xt[:], in_=xt[:], func=AF.Identity, bias=magic_b[:],
                    scale=inv_sx[:],
                )
                xq = xqp.tile([128, c], BF16, tag="xq")
                nc.vector.tensor_scalar_add(out=xq[:], in0=xt[:], scalar1=-MAGIC)
                if pe_transpose:
                    # prologue only: PE is idle and the post-AllReduce path
                    # must not queue behind the ring
                    for cq in range(ct // 4):
                        pt = psT.tile([128, 512], F32, tag="psT")
                        for k in range(4):
                            cb = cq * 4 + k
                            nc.tensor.matmul(
                                pt[:, k * 128 : (k + 1) * 128],
                                lhsT=xq[:, cb * 128 : (cb + 1) * 128],
                                rhs=ident[:], start=True, stop=True,
                            )
                        nc.vector.tensor_copy(
                            out=xqT[:, cq * 4 : (cq + 1) * 4,
                                    t8 * 128 : (t8 + 1) * 128],
                            in_=pt[:].rearrange("p (a b) -> p a b", a=4),
                        )
                else:
                    nc.sync.dma_start(out=xq_dram[m0 : m0 + 128, :], in_=xq[:])
            if not pe_transpose:
                nc.sync.dma_start_transpose(
                    out=xqT[:],
                    in_=xq_dram[mc * CH : (mc + 1) * CH, :],
                )
            return xqT

        # Prologue interleave on the sync ring: chunk-0 x loads + quant
        # (PE-transposed, gated only by AR_x), then the w1 strip pipeline
        # (ring traffic is just the f32 loads; transposes on the idle PE).
        xqT0 = x_quant_chunk(0, pe_transpose=True)
        for rb in range(ht):
            w1_strip_pipeline(rb)
        w1pools.close()

        w2pools = ExitStack()
        w2s = w2pools.enter_context(tc.tile_pool(name="w2s", bufs=1))
        w2sr = w2pools.enter_context(tc.tile_pool(name="w2sr", bufs=2))
        w2qf = w2pools.enter_context(tc.tile_pool(name="w2qf", bufs=2))
        w2qb = w2pools.enter_context(tc.tile_pool(name="w2qb", bufs=2))
        psHs = ExitStack()
        psH = psHs.enter_context(tc.tile_pool(
            name="psH", bufs=(8 if n_ms == 2 else 3 * n_ms), space="PSUM"))

        # pipelined emission (full config): next-chunk x prep and w2-quant
        # pieces are interleaved into the jb loop so their ring traffic and
        # ACT work spread across the chunk instead of bunching at chunk
        # boundaries.
        pipelined = ht >= 32 and CH // 128 <= 8
        n_w2q = h // hw2
        w2_piece_slots = {}
        if pipelined:
            slots = []
            for mcs in range(n_chunks_a):
                for jbs in range(16 if mcs == 0 else 2, ht, 2):
                    slots.append((mcs, jbs))
            assert len(slots) >= n_w2_pieces
            for i in range(n_w2_pieces):
                w2_piece_slots[slots[i]] = i

        def w2_quant_piece(pc):
            inv_sw2 = sw2_state["inv_sw2"]
            ob, q = pc // n_w2q, pc % n_w2q
            wt = w2qf.tile([128, hw2], F32, tag="w2qf")
            nc.sync.dma_start(
                out=wt[:],
                in_=w2_in[ob * 128 : (ob + 1) * 128, q * hw2 : (q + 1) * hw2],
            )
            nc.scalar.activation(
                out=wt[:], in_=wt[:], func=AF.Identity,
                bias=magic_b[:], scale=inv_sw2[:],
            )
            wq = w2qb.tile([128, hw2], BF16, tag="w2qb")
            nc.vector.tensor_scalar_add(out=wq[:], in0=wt[:], scalar1=-MAGIC)
            nc.sync.dma_start(
                out=w2q_dram[ob * 128 : (ob + 1) * 128,
                             q * hw2 : (q + 1) * hw2],
                in_=wq[:],
            )

        w2_stack = ExitStack()
        w2qTs = None
        if True:
            xqT_next = None
            for mc in range(n_chunks_a):
                if pipelined and mc == n_chunks_a - 1:
                    # last chunk: w2pools/inner1 space is free now — land the
                    # w2qT transpose-loads there so they ride the ring during
                    # this chunk's matmuls, not the transition
                    w2pools.close()
                    inner1.close()
                    w2qT_pool = w2_stack.enter_context(
                        tc.tile_pool(name="w2qT", bufs=1)
                    )
                    w2qTs = w2qT_pool.tile([128, ht, c], BF16, tag="w2qTs")
                    for jg in range(ht // 4):
                        nc.sync.dma_start_transpose(
                            out=w2qTs[:, jg * 4 : (jg + 1) * 4, :],
                            in_=w2q_dram[:, jg * 512 : (jg + 1) * 512],
                        )
                # --- sharded w2 amax scan + its AllReduce, during chunk 0 ---
                if mc == (0 if pipelined else min(1, n_chunks_a - 2)):
                    w2s_rows = c // n_cores
                    for rb in range((w2s_rows + 127) // 128):
                        nrow = min(128, w2s_rows - rb * 128)
                        wt = w2s.tile([128, h], F32, tag="w2s")
                        nc.sync.dma_start(
                            out=wt[:nrow, :],
                            in_=w2s_in[rb * 128 : rb * 128 + nrow, :],
                        )
                        wr = w2sr.tile([128, 1], F32, tag="w2sr")
                        nc.vector.tensor_reduce(
                            out=wr[:nrow], in_=wt[:nrow, :],
                            axis=mybir.AxisListType.X,
                            op=ALU.max, apply_absolute_value=True,
                        )
                        nc.vector.tensor_tensor(
                            out=wmax2[:nrow], in0=wmax2[:nrow], in1=wr[:nrow],
                            op=ALU.max
                        )
                    w2max_r = _preduce(wmax2, psH, "psH")
                    nc.gpsimd.dma_start(out=arw_in[:], in_=w2max_r[:])
                    nc.gpsimd.collective_compute(
                        "AllReduce", ALU.max, replica_groups=groups,
                        ins=[arw_in.opt()], outs=[arw_out.opt()],
                    )
                if "inv_sw2" not in sw2_state and (
                    (pipelined and mc == 0) or (not pipelined and mc in w2_quant_chunks)
                ):
                    sw2_state["sw2"], sw2_state["inv_sw2"] = _derive(
                        arw_out, "w2"
                    )
                if not pipelined and mc in w2_quant_chunks:
                    qbase = sum(v for k, v in w2_quant_chunks.items() if k < mc)
                    for k in range(w2_quant_chunks[mc]):
                        w2_quant_piece(qbase + k)

                # --- x quant for this chunk ---
                if mc == 0:
                    xqT = xqT0
                elif pipelined:
                    xqT = xqT_next
                else:
                    xqT = x_quant_chunk(mc)

                # next-chunk x prep state (pipelined emission)
                nt8 = CH // 128
                x_next = {}

                # --- matmul over j blocks; gelu; amax; store h.T fp16 ---
                for jb in range(ht):
                    if pipelined and mc + 1 < n_chunks_a:
                        # staggered load->quant->store keeps xa live <= 4
                        if jb % 2 == 0 and jb < 2 * nt8:
                            t8 = jb // 2
                            m0 = (mc + 1) * CH + t8 * 128
                            xt = xa.tile([128, c], F32, tag="xa")
                            nc.sync.dma_start(
                                out=xt[:], in_=x_in[m0 : m0 + 128, :]
                            )
                            x_next[t8] = xt
                        if jb % 2 == 1 and 5 <= jb < 5 + 2 * nt8:
                            t8 = (jb - 5) // 2
                            xt = x_next.pop(t8)
                            m0 = (mc + 1) * CH + t8 * 128
                            nc.scalar.activation(
                                out=xt[:], in_=xt[:], func=AF.Identity,
                                bias=magic_b[:], scale=inv_sx[:],
                            )
                            xq = xqp.tile([128, c], BF16, tag="xq")
                            nc.vector.tensor_scalar_add(
                                out=xq[:], in0=xt[:], scalar1=-MAGIC
                            )
                            nc.sync.dma_start(
                                out=xq_dram[m0 : m0 + 128, :], in_=xq[:]
                            )
                        elif jb == 6 + 2 * nt8:
                            xqT_next = xqtp.tile(
                                [128, ct, CH], BF16, tag="xqT",
                                name=f"xqT{mc + 1}",
                            )
                            nc.sync.dma_start_transpose(
                                out=xqT_next[:],
                                in_=xq_dram[(mc + 1) * CH : (mc + 2) * CH, :],
                            )
                    if pipelined and (mc, jb) in w2_piece_slots:
                        w2_quant_piece(w2_piece_slots[(mc, jb)])

                    phs = [
                        psH.tile([128, 512], F32, tag="psH", name=f"psH{mc}_{jb}_{i}")
                        for i in range(n_ms)
                    ]
                    prev = None
                    for cb in range(ct):
                        for ms in range(n_ms):
                            mmi = nc.tensor.matmul(
                                phs[ms][:],
                                lhsT=w1qT_ap(cb, jb),
                                rhs=xqT[:, cb, ms * 512 : (ms + 1) * 512],
                                start=(cb == 0),
                                stop=(cb == ct - 1),
                            )
                            if prev is not None:
                                _add_dep(mmi.ins, prev.ins, sync=False,
                                         reason="ldw-order")
                            prev = mmi
                    for ms in range(n_ms):
                        ph = phs[ms]
                        g = gs.tile([128, 512], F16, tag="gs")
                        if gelu == "Erf":
                            hh = gs.tile([128, 512], F32, tag="gh")
                            nc.scalar.activation(
                                out=hh[:], in_=ph[:], func=AF.Identity,
                                bias=b1_sb[:, jb : jb + 1], scale=sxw1[:],
                            )
                            e = gs.tile([128, 512], F32, tag="ge")
                            nc.scalar.activation(
                                out=e[:], in_=hh[:], func=AF.Erf, bias=0.0,
                                scale=float(1.0 / np.sqrt(2.0)),
                            )
                            nc.vector.tensor_scalar(
                                out=e[:], in0=e[:], scalar1=0.5, scalar2=0.5,
                                op0=ALU.mult, op1=ALU.add,
                            )
                            nc.vector.tensor_tensor(
                                out=g[:], in0=e[:], in1=hh[:], op=ALU.mult
                            )
                        else:
                            nc.scalar.activation(
                                out=g[:], in_=ph[:], func=getattr(AF, gelu),
                                bias=b1_sb[:, jb : jb + 1], scale=sxw1[:],
                            )
                        r = gr.tile([128, 1], F32, tag="gr")
                        nc.vector.tensor_reduce(
                            out=r[:], in_=g[:], axis=mybir.AxisListType.X,
                            op=ALU.max, apply_absolute_value=True,
                        )
                        nc.vector.tensor_tensor(
                            out=hmax[:], in0=hmax[:], in1=r[:], op=ALU.max
                        )
                        m0 = mc * CH + ms * 512
                        nc.sync.dma_start(
                            out=h_dram[jb * 128 : (jb + 1) * 128, m0 : m0 + 512],
                            in_=g[:],
                        )

        psHs.close()
        w1_stack.close()
        sw2 = sw2_state["sw2"]

        # ---------- h scale: AllReduce ----------
        with tc.tile_pool(name="psRh", bufs=2, space="PSUM") as psRh:
            hmax_r = _preduce(hmax, psRh, "hm")
        nc.gpsimd.dma_start(out=arh_in[:], in_=hmax_r[:])
        nc.gpsimd.collective_compute(
            "AllReduce", ALU.max, replica_groups=groups,
            ins=[arh_in.opt()], outs=[arh_out.opt()],
        )

        sh, inv_sh = _derive(arh_out, "h")
        shw2 = scal.tile([128, 1], F32)
        nc.vector.tensor_tensor(out=shw2[:], in0=sh[:], in1=sw2[:], op=ALU.mult)

        # ---------- phase B: y = hq.T.T @ w2q.T * (sh*sw2) + b2 ----------
        n_chunk = rows // 512
        if w2qTs is None:
            # non-pipelined fallback: materialize w2qT at the transition
            w2pools.close()
            inner1.close()
            w2qT_pool = w2_stack.enter_context(tc.tile_pool(name="w2qT", bufs=1))
            w2qTs = w2qT_pool.tile([128, ht, c], BF16, tag="w2qTs")
            for jg in range(ht // 4):
                nc.sync.dma_start_transpose(
                    out=w2qTs[:, jg * 4 : (jg + 1) * 4, :],
                    in_=w2q_dram[:, jg * 512 : (jg + 1) * 512],
                )
        with tc.tile_pool(
            name="hb", bufs=6
        ) as hb, tc.tile_pool(name="hf", bufs=3) as hf, tc.tile_pool(
            name="hqt", bufs=2
        ) as hqtp, tc.tile_pool(name="ys", bufs=4) as ys, tc.tile_pool(
            name="psY", bufs=3 * (c // 512), space="PSUM"
        ) as psY:
            def hqT_prep(mc):
                hqT = hqtp.tile([128, ht * 512], BF16, tag="hqT")
                for jb in range(ht):
                    th = hb.tile([128, 512], F16, tag="hb")
                    nc.sync.dma_start(
                        out=th[:],
                        in_=h_dram[jb * 128 : (jb + 1) * 128,
                                   mc * 512 : (mc + 1) * 512],
                    )
                    tf = hf.tile([128, 512], F32, tag="hf")
                    nc.scalar.activation(
                        out=tf[:], in_=th[:], func=AF.Identity, bias=magic_b[:],
                        scale=inv_sh[:],
                    )
                    nc.vector.tensor_scalar_add(
                        out=hqT[:, jb * 512 : (jb + 1) * 512], in0=tf[:],
                        scalar1=-MAGIC,
                    )
                return hqT

            # chunk-0 h loads: their dequant hides under the h-scale
            # AllReduce (w2qT already streamed during the last phase-A chunk)
            hqT0 = hqT_prep(0)

            for mc in range(n_chunk):
                hqT = hqT0 if mc == 0 else hqT_prep(mc)
                n_ob = c // 512
                for ms in range(4):
                    pys = [
                        psY.tile([128, 512], F32, tag="psY", name=f"psY{mc}_{ms}_{i}")
                        for i in range(n_ob)
                    ]
                    prev = None
                    for jb in range(ht):
                        for ob in range(n_ob):
                            mmi = nc.tensor.matmul(
                                pys[ob][:],
                                lhsT=hqT[:, jb * 512 + ms * 128 :
                                         jb * 512 + (ms + 1) * 128],
                                rhs=w2qTs[:, jb, ob * 512 : (ob + 1) * 512],
                                start=(jb == 0),
                                stop=(jb == ht - 1),
                            )
                            if prev is not None:
                                _add_dep(mmi.ins, prev.ins, sync=False,
                                         reason="ldw-order")
                            prev = mmi
                    for ob in range(n_ob):
                        yt = ys.tile([128, 512], F32, tag="ys")
                        nc.vector.scalar_tensor_tensor(
                            out=yt[:], in0=pys[ob][:], scalar=shw2[:],
                            in1=b2_b[:, ob * 512 : (ob + 1) * 512],
                            op0=ALU.mult, op1=ALU.add,
                        )
                        m0 = mc * 512 + ms * 128
                        nc.sync.dma_start(
                            out=y_out[m0 : m0 + 128, ob * 512 : (ob + 1) * 512],
                            in_=yt[:],
                        )

        w2_stack.close()
        pa0.close()

    if split_waits:
        _split_matmul_waits(nc)
        _dedup_ldweights(nc)
    return nc


_CACHED = {}


def _get_nc(rows, c, h, n_cores, gelu):
    key = (rows, c, h, n_cores, gelu)
    if key not in _CACHED:
        _CACHED[key] = build_nc(rows=rows, c=c, h=h, n_cores=n_cores, gelu=gelu)
    return _CACHED[key]


def run(inputs, trace=False, gelu="Gelu", n_cores=N_CORES):
    x = np.asarray(inputs["x"], np.float32)
    w1 = np.ascontiguousarray(np.asarray(inputs["w1"], np.float32))
    b1 = np.ascontiguousarray(np.asarray(inputs["b1"], np.float32))
    w2 = np.ascontiguousarray(np.asarray(inputs["w2"], np.float32))
    b2 = np.ascontiguousarray(np.asarray(inputs["b2"], np.float32))
    b_, s_, c_ = x.shape
    h_ = w1.shape[0]
    x2d = np.ascontiguousarray(x.reshape(-1, c_))
    rows = x2d.shape[0] // n_cores
    nc = _get_nc(rows, c_, h_, n_cores, gelu)
    w1r = h_ // n_cores
    w2r = c_ // n_cores
    in_maps = [
        {
            "x": np.ascontiguousarray(x2d[i * rows : (i + 1) * rows]),
            "w1": w1,
            "b1": b1,
            "w2": w2,
            "b2": b2,
            "w1s": np.ascontiguousarray(w1[i * w1r : (i + 1) * w1r]),
            "w2s": np.ascontiguousarray(w2[i * w2r : (i + 1) * w2r]),
        }
        for i in range(n_cores)
    ]
    res = run_bass_kernel_spmd(nc, in_maps, list(range(n_cores)), trace=trace)
    y2d = np.concatenate([r["y"] for r in res.results], axis=0)
    return y2d.reshape(b_, s_, c_).astype(np.float32), res


def kernel(x, w1, b1, w2, b2):
    y, _ = run({"x": x, "w1": w1, "b1": b1, "w2": w2, "b2": b2})
    return y


# revision 63
# speedup vs baseline: 1.0441x; 1.0441x over previous
"""Trainium2 Bass kernel for a fake-quantized MLP (qlinear -> gelu -> qlinear).

Reference semantics (B,S,C,H = 32,1024,1024,4096):
    x2d = x.reshape(-1, C)
    h   = round(x2d/sx) @ round(w1/sw1).T * (sx*sw1) + b1 ;  s = max(amax,eps)/127
    g   = gelu(h, exact erf)
    y   = round(g/sh) @ round(w2/sw2).T * (sh*sw2) + b2

Strategy: data-parallel over rows across 8 cores.  Quantized ints fit
exactly in bf16, so matmuls run at full bf16 rate with exact fp32 PSUM
accumulation.  Weight scales need no collective (every core scans the full
weights); the two activation scales (x, gelu output) use tiny
AllReduce(max) collectives.

v2 layout changes vs the first working version:
  - all quantized-tile transposes go through the DMA xbar (SBUF->SBUF for
    x/w1 block transposes, DRAM->SBUF bulk transpose-load for w2) instead
    of PE identity matmuls + ACT copies: keeps PE for real matmuls and
    unclogs the ACT engine that previously gated w1 prep.
  - gelu output h is staged to DRAM in fp16 (not fp32): halves the
    mid-kernel HBM traffic; fp16 rounding of g only perturbs the final
    int8 requant on ~0.1% boundary cases (noise ~1e-3 of output max).
  - w2 amax scan + quant + store of quantized-natural w2q (bf16, DRAM)
    are trickled through phase A; at the phase A->B transition w2qT is
    materialized with 8 bulk DMA transpose-loads that hide under the
    h-scale AllReduce.
  - x amax scan is emitted first so its AllReduce launches as early as
    possible; w1 scan + quant pipeline fills the AllReduce latency.
"""

import sys

import numpy as np

try:
    import concourse.bass as bass
except ImportError:  # pragma: no cover
    sys.path.insert(0, "/opt/trn_rl_repo")
    import concourse.bass as bass

import concourse.mybir as mybir
from contextlib import ExitStack
import concourse.tile as tile
from concourse.bass_utils import run_bass_kernel_spmd

from concourse.bass import _add_dep_helper as _add_dep

F32 = mybir.dt.float32
F16 = mybir.dt.float16
BF16 = mybir.dt.bfloat16
AF = mybir.ActivationFunctionType
ALU = mybir.AluOpType

QP = 127.0
EPS = 1e-6
MAGIC = 12582912.0  # 1.5 * 2**23: fp32 round-to-nearest-even integer trick

# full problem shapes
B, S, C, H = 32, 1024, 1024, 4096
N_CORES = 8


def _split_matmul_waits(nc):
    """This toolchain's walrus codegen allows only ONE sync-wait slot per
    lowered instruction (Matmult waits all land on its LDWEIGHTS since
    --enable-ldw-opt=false; queue DMAs use a single-slot DIRECT2D struct).
    Peel extra waits onto same-engine NoOps inserted just before, except for
    framework-generated drain/barrier instructions which support many."""
    n_split = 0
    for f in nc.m.functions:
        for bb in f.blocks:
            insts = bb.instructions
            out = []
            changed = False
            for inst in insts:
                si = getattr(inst, "sync_info", None)
                if si is not None and si.on_wait and len(si.on_wait) > 1:
                    waits = list(si.on_wait)
                    for k, w in enumerate(waits[:-1]):
                        nop = mybir.InstNoOp(
                            name=f"{inst.name}-wsplit{k}", ins=[], outs=[]
                        )
                        nop.engine = inst.engine
                        nop.sync_info = mybir.SyncInfo(
                            on_wait=[w], on_update=[]
                        )
                        out.append(nop)
                    inst.sync_info = mybir.SyncInfo(
                        on_wait=[waits[-1]], on_update=list(si.on_update or [])
                    )
                    n_split += 1
                    changed = True
                out.append(inst)
            if changed:
                bb.instructions = out
    return n_split


def _dedup_ldweights(nc):
    """Tile legalization emits explicit Ldweights+Matmult pairs, and walrus
    runs with --enable-ldw-opt=false, so every matmul re-streams its
    stationary operand (128 extra PE cycles on a 512-cycle matmul).  Drop an
    Ldweights whose weights AP is identical to the previous one on the PE
    stream (the PE array still holds that stationary); keep its semaphore
    effects on a NoOp."""
    n = 0
    for f in nc.m.functions:
        for bb in f.blocks:
            insts = bb.instructions
            out = []
            last_key = None
            changed = False
            for inst in insts:
                if isinstance(inst, mybir.InstLdweights):
                    key = str(inst.ins[0])
                    if key == last_key:
                        si = getattr(inst, "sync_info", None)
                        if si is not None and (si.on_wait or si.on_update):
                            nop = mybir.InstNoOp(
                                name=inst.name + "-lw", ins=[], outs=[]
                            )
                            nop.engine = inst.engine
                            nop.sync_info = si
                            out.append(nop)
                        n += 1
                        changed = True
                        continue
                    last_key = key
                elif isinstance(inst, mybir.InstMatmult):
                    if inst.is_transpose or getattr(inst, "ldweights", None):
                        last_key = None
                out.append(inst)
            if changed:
                bb.instructions = out
    return n


def build_nc(rows=4096, c=C, h=H, n_cores=N_CORES, gelu="Gelu", split_waits=True):
    """Build the per-core SPMD Bass program.

    rows: rows of x2d handled by each core.
    gelu: "Gelu" (HW ACT table), "Erf" (x*(0.5*erf(x/sqrt2)+0.5)),
          "Identity" (for simulator runs; CoreSim lacks Gelu/Erf).
    """
    assert rows % 512 == 0 and c % 512 == 0 and h % 512 == 0
    nc = bass.Bass()

    x_in = nc.dram_tensor("x", [rows, c], F32, kind="ExternalInput")
    w1_in = nc.dram_tensor("w1", [h, c], F32, kind="ExternalInput")
    b1_in = nc.dram_tensor("b1", [h], F32, kind="ExternalInput")
    w2_in = nc.dram_tensor("w2", [c, h], F32, kind="ExternalInput")
    b2_in = nc.dram_tensor("b2", [c], F32, kind="ExternalInput")
    # per-core weight shards (sliced in run()) so the amax scans are
    # 1/n_cores the traffic; the partial maxes AllReduce to the global max
    w1s_in = nc.dram_tensor("w1s", [h // n_cores, c], F32, kind="ExternalInput")
    w2s_in = nc.dram_tensor("w2s", [c // n_cores, h], F32, kind="ExternalInput")
    y_out = nc.dram_tensor("y", [rows, c], F32, kind="ExternalOutput")

    ct = c // 128   # c in 128-blocks
    ht = h // 128   # h in 128-blocks
    groups = [list(range(n_cores))]

    with tile.TileContext(nc) as tc, ExitStack() as top:
        consts = top.enter_context(tc.tile_pool(name="consts", bufs=1))
        scal = top.enter_context(tc.tile_pool(name="scal", bufs=1))
        dram = top.enter_context(tc.tile_pool(name="dram", bufs=1, space="DRAM"))

        ident_f = consts.tile([128, 128], F32)
        from concourse import masks
        masks.make_identity(nc, ident_f[:])
        ident = consts.tile([128, 128], BF16)
        masks.make_identity(nc, ident[:])

        # b1 as (128, ht): b1_sb[p, jb] = b1[jb*128 + p]
        b1_sb = consts.tile([128, ht], F32)
        nc.sync.dma_start(
            out=b1_sb[:], in_=b1_in.ap().rearrange("(a b) -> b a", b=128)
        )

        magic_b = consts.tile([128, 1], F32)
        nc.vector.memset(magic_b[:], MAGIC)

        # b2 broadcast early: its 128x4KB broadcast packets should ride the
        # idle prologue ring, not the busy phase-B transition
        b2_b = consts.tile([128, c], F32)
        nc.sync.dma_start(
            out=b2_b[:],
            in_=b2_in.ap().rearrange("(o a) -> o a", o=1).to_broadcast((128, c)),
        )

        # h scratch in DRAM, transposed + fp16: (h, rows)
        h_dram = dram.tile([h, rows], F16)
        # quantized-natural staging (bf16) for the bulk DMA transpose-loads.
        # Per-block SBUF->SBUF xbar transposes degrade into 256B packets on a
        # single HWDGE ring (measured ~25x DMA throughput collapse); bulk
        # DRAM->SBUF transpose-loads with contiguous >=1KB source rows run at
        # ~280GB/s.  x and w2 bounce through DRAM; w1 (prologue critical
        # path, PE idle anyway) is transposed on the PE instead to keep
        # 17MB off the sync ring.
        xq_dram = dram.tile([rows, c], BF16, tag="xqd")
        w2q_dram = dram.tile([c, h], BF16, tag="w2qd")
        # collective bounce buffers (DRAM, non-IO).  arx carries [xmax, w1max]
        # in one 2-element AllReduce; arw carries the sharded w2max.
        arx_in = dram.tile([2, 1], F32, tag="arxi")
        arx_out = dram.tile([2, 1], F32, tag="arxo")
        arw_in = dram.tile([1, 1], F32, tag="arwi")
        arw_out = dram.tile([1, 1], F32, tag="arwo")
        arh_in = dram.tile([1, 1], F32, tag="arhi")
        arh_out = dram.tile([1, 1], F32, tag="arho")

        def _preduce(acc, psR, tag):
            """[128,1] partition-max -> [1,1] via PE transpose + DVE reduce."""
            pt = psR.tile([1, 128], F32, tag=tag)
            nc.tensor.matmul(
                pt[:], lhsT=acc[:], rhs=ident_f[:], start=True, stop=True
            )
            out = scal.tile([1, 1], F32, name=tag + "_r")
            nc.vector.tensor_reduce(
                out=out[:], in_=pt[:], axis=mybir.AxisListType.X, op=ALU.max
            )
            return out

        def _derive(bcast_src_dram, name):
            b = scal.tile([128, 1], F32, name=name + "_b")
            nc.sync.dma_start(out=b[:], in_=bcast_src_dram.to_broadcast((128, 1)))
            s = scal.tile([128, 1], F32, name="s_" + name)
            nc.vector.tensor_scalar(
                out=s[:], in0=b[:], scalar1=EPS, scalar2=float(1.0 / QP),
                op0=ALU.max, op1=ALU.mult,
            )
            inv = scal.tile([128, 1], F32, name="inv_" + name)
            nc.vector.reciprocal(out=inv[:], in_=s[:])
            return s, inv

        # ---------- phase 0: x amax scan FIRST (its AllReduce is the long
        # pole: ~50us mesh latency).  512-row tiles, DMA-bound. ----------
        xmax = scal.tile([128, 1], F32)
        nc.vector.memset(xmax[:], 0.0)
        wmax1 = scal.tile([128, 1], F32)
        nc.vector.memset(wmax1[:], 0.0)
        wmax2 = scal.tile([128, 1], F32)
        nc.vector.memset(wmax2[:], 0.0)

        with tc.tile_pool(name="scan", bufs=4) as scanp, tc.tile_pool(
            name="scant", bufs=4
        ) as scant, tc.tile_pool(name="psR", bufs=2, space="PSUM") as psR0:
            for mb in range(rows // 512):
                t = scanp.tile([128, c * 4], F32, tag="sc")
                nc.sync.dma_start(
                    out=t[:].rearrange("b (a c) -> b a c", a=4),
                    in_=x_in[mb * 512 : (mb + 1) * 512, :].rearrange(
                        "(a b) c -> b a c", b=128
                    ),
                )
                r = scant.tile([128, 1], F32, tag="scr")
                nc.vector.tensor_reduce(
                    out=r[:], in_=t[:], axis=mybir.AxisListType.X, op=ALU.max,
                    apply_absolute_value=True,
                )
                nc.vector.tensor_tensor(
                    out=xmax[:], in0=xmax[:], in1=r[:], op=ALU.max
                )
            xmax_r = _preduce(xmax, psR0, "xm")
            nc.gpsimd.dma_start(out=arx_in[0:1, :], in_=xmax_r[:])

            # sharded w1 amax scan: this core's 1/n_cores slice only; the
            # partial max rides the same AllReduce as xmax (2 elements).
            # MUST stay on the sync HWDGE ring: the scalar ring concurrently
            # (in a NEFF that also issues xbar DMA transposes) crashes the
            # device; SWDGE/gpsimd takes ~1ms to software-generate the
            # descriptors for these strided tiles.
            w1s_rows = h // n_cores
            for rb in range((w1s_rows + 511) // 512):
                nrow = min(512, w1s_rows - rb * 512)
                nb = nrow // 128
                t = scanp.tile([128, c * nb], F32, tag="sc")
                nc.sync.dma_start(
                    out=t[:, : c * nb].rearrange("b (a c) -> b a c", a=nb),
                    in_=w1s_in[rb * 512 : rb * 512 + nrow, :].rearrange(
                        "(a b) c -> b a c", b=128
                    ),
                )
                r = scant.tile([128, 1], F32, tag="scr")
                nc.vector.tensor_reduce(
                    out=r[:], in_=t[:, : c * nb], axis=mybir.AxisListType.X,
                    op=ALU.max, apply_absolute_value=True,
                )
                nc.vector.tensor_tensor(
                    out=wmax1[:], in0=wmax1[:], in1=r[:], op=ALU.max
                )
            w1max_r = _preduce(wmax1, psR0, "w1m")
            nc.gpsimd.dma_start(out=arx_in[1:2, :], in_=w1max_r[:])
            nc.gpsimd.collective_compute(
                "AllReduce", ALU.max, replica_groups=groups,
                ins=[arx_in.opt()], outs=[arx_out.opt()],
            )

        sw1, inv_sw1 = _derive(arx_out[1:2, :], "w1")
        sx, inv_sx = _derive(arx_out[0:1, :], "x")
        sxw1 = scal.tile([128, 1], F32)
        nc.vector.tensor_tensor(out=sxw1[:], in0=sx[:], in1=sw1[:], op=ALU.mult)

        hmax = scal.tile([128, 1], F32)
        nc.vector.memset(hmax[:], 0.0)

        # ---------- w1 quant -> w1q_dram -> bulk transpose-loads ----------
        # w1qT_all layout: [128(p=c%128), jg(h/512), cb(ct), 512(j within jg)]
        # so matmul lhsT for (cb, jb) = [:, jb//4, cb, (jb%4)*128:+128].
        # Pipelined per 512-row group; emission order on the single sync ring
        # is chosen so the operands of the first matmuls land first:
        # x chunk-0 f32 loads, w1 jg=0, x chunk-0 quant+transpose, w1 jg>0.
        n_jg1 = h // 512
        w1_stack = ExitStack()
        w1qT_pool = w1_stack.enter_context(
            tc.tile_pool(name="w1qT", bufs=1, side="right")
        )
        w1qT_all = w1qT_pool.tile([128, n_jg1, ct, 512], BF16, tag="w1qT")

        def w1qT_ap(cb, jb):
            return w1qT_all[:, jb // 4, cb, (jb % 4) * 128 : (jb % 4 + 1) * 128]

        # ---------- phase A: h.T = gelu(w1q @ xq.T * (sx*sw1) + b1) -------
        CH = min(1024, rows // 2)
        n_ms = CH // 512
        n_chunks_a = rows // CH
        # w2 plan: sharded amax scan + AllReduce launched during chunk 0;
        # quant+store pieces ([128, h/2]) trickled through the last two
        # chunks (sw2 only needs to be ready by then).
        assert n_chunks_a >= 2
        hw2 = min(1024, h)
        n_w2_pieces = ct * (h // hw2)
        if n_chunks_a >= 4:
            w2_quant_chunks = {2: n_w2_pieces // 2,
                               3: n_w2_pieces - n_w2_pieces // 2}
        else:
            w2_quant_chunks = {1: n_w2_pieces}
        sw2_state = {}

        # Pool lifetime plan (SBUF pools are a LIFO stack; PSUM has its own):
        #   pa0 (gs, gr, xqt)        — open until the very end
        #   inner1 (xa, xq)          — closes after chunk-(n-2) emission
        #   w1pools (w1f, w1q)       — closes after the prologue
        #   w2pools (w2 scan/quant)  — closes after the last w2 piece
        #   w2qTp                    — opens at chunk-(n-1): its tiles land in
        #                              the space w2pools/inner1 just freed, so
        #                              the w2qT transpose-loads ride the ring
        #                              DURING the last chunk instead of
        #                              serializing into the transition.
        pa0 = ExitStack()
        gs = pa0.enter_context(tc.tile_pool(name="gs", bufs=12))
        gr = pa0.enter_context(tc.tile_pool(name="gr", bufs=8))
        xqtp = pa0.enter_context(tc.tile_pool(name="xqt", bufs=2))
        inner1 = ExitStack()
        xa = inner1.enter_context(tc.tile_pool(name="xa", bufs=5))
        xqp = inner1.enter_context(tc.tile_pool(name="xq", bufs=2))
        w1pools = ExitStack()
        w1f = w1pools.enter_context(tc.tile_pool(name="w1f", bufs=6))
        w1qp = w1pools.enter_context(tc.tile_pool(name="w1q", bufs=6))
        psT = w1pools.enter_context(tc.tile_pool(name="psT", bufs=4, space="PSUM"))

        def w1_strip_pipeline(rb):
            """load strip -> quantize -> PE-transpose 128x128 blocks into
            w1qT_all (batched 4 blocks per PSUM bank, one DVE copy each)."""
            wt = w1f.tile([128, c], F32, tag="w1f")
            nc.sync.dma_start(
                out=wt[:], in_=w1_in[rb * 128 : (rb + 1) * 128, :]
            )
            nc.scalar.activation(
                out=wt[:], in_=wt[:], func=AF.Identity, bias=magic_b[:],
                scale=inv_sw1[:],
            )
            wq = w1qp.tile([128, c], BF16, tag="w1q")
            nc.vector.tensor_scalar_add(out=wq[:], in0=wt[:], scalar1=-MAGIC)
            jg, jsub = rb // 4, (rb % 4) * 128
            for cq in range(ct // 4):
                pt = psT.tile([128, 512], F32, tag="psT")
                for k in range(4):
                    cb = cq * 4 + k
                    nc.tensor.matmul(
                        pt[:, k * 128 : (k + 1) * 128],
                        lhsT=wq[:, cb * 128 : (cb + 1) * 128],
                        rhs=ident[:], start=True, stop=True,
                    )
                nc.vector.tensor_copy(
                    out=w1qT_all[:, jg, cq * 4 : (cq + 1) * 4,
                                 jsub : jsub + 128],
                    in_=pt[:].rearrange("p (a b) -> p a b", a=4),
                )

        def x_quant_chunk(mc, pe_transpose=False):
            xqT = xqtp.tile([128, ct, CH], BF16, tag="xqT", name=f"xqT{mc}")
            for t8 in range(CH // 128):
                m0 = mc * CH + t8 * 128
                xt = xa.tile([128, c], F32, tag="xa")
                nc.sync.dma_start(out=xt[:], in_=x_in[m0 : m0 + 128, :])
                nc.scalar.activation(
                    out=xt[:], in_=xt[:], func=AF.Identity, bias=magic_b[:],
                    scale=inv_sx[:],
                )
                xq = xqp.tile([128, c], BF16, tag="xq")
                nc.vector.tensor_scalar_add(out=xq[:], in0=xt[:], scalar1=-MAGIC)
                if pe_transpose:
                    # prologue only: PE is idle and the post-AllReduce path
                    # must not queue behind the ring
                    for cq in range(ct // 4):
                        pt = psT.tile([128, 512], F32, tag="psT")
                        for k in range(4):
                            cb = cq * 4 + k
                            nc.tensor.matmul(
                                pt[:, k * 128 : (k + 1) * 128],
                                lhsT=xq[:, cb * 128 : (cb + 1) * 128],
                                rhs=ident[:], start=True, stop=True,
                            )
                        nc.vector.tensor_copy(
                            out=xqT[:, cq * 4 : (cq + 1) * 4,
                                    t8 * 128 : (t8 + 1) * 128],
                            in_=pt[:].rearrange("p (a b) -> p a b", a=4),
                        )
                else:
                    nc.sync.dma_start(out=xq_dram[m0 : m0 + 128, :], in_=xq[:])
            if not pe_transpose:
                nc.sync.dma_start_transpose(
                    out=xqT[:],
                    in_=xq_dram[mc * CH : (mc + 1) * CH, :],
                )
            return xqT

        # Prologue interleave on the sync ring: chunk-0 x loads + quant
        # (PE-transposed, gated only by AR_x), then the w1 strip pipeline
        # (ring traffic is just the f32 loads; transposes on the idle PE).
        xqT0 = x_quant_chunk(0, pe_transpose=True)
        for rb in range(ht):
            w1_strip_pipeline(rb)
        w1pools.close()

        w2pools = ExitStack()
        w2s = w2pools.enter_context(tc.tile_pool(name="w2s", bufs=1))
        w2sr = w2pools.enter_context(tc.tile_pool(name="w2sr", bufs=2))
        w2qf = w2pools.enter_context(tc.tile_pool(name="w2qf", bufs=2))
        w2qb = w2pools.enter_context(tc.tile_pool(name="w2qb", bufs=2))
        psHs = ExitStack()
        psH = psHs.enter_context(tc.tile_pool(
            name="psH", bufs=(8 if n_ms == 2 else 3 * n_ms), space="PSUM"))

        # pipelined emission (full config): next-chunk x prep and w2-quant
        # pieces are interleaved into the jb loop so their ring traffic and
        # ACT work spread across the chunk instead of bunching at chunk
        # boundaries.
        pipelined = ht >= 32 and CH // 128 <= 8
        n_w2q = h // hw2
        w2_piece_slots = {}
        if pipelined:
            slots = []
            for mcs in range(n_chunks_a):
                for jbs in range(16 if mcs == 0 else 2, ht, 2):
                    slots.append((mcs, jbs))
            assert len(slots) >= n_w2_pieces
            for i in range(n_w2_pieces):
                w2_piece_slots[slots[i]] = i

        def w2_quant_piece(pc):
            inv_sw2 = sw2_state["inv_sw2"]
            ob, q = pc // n_w2q, pc % n_w2q
            wt = w2qf.tile([128, hw2], F32, tag="w2qf")
            nc.sync.dma_start(
                out=wt[:],
                in_=w2_in[ob * 128 : (ob + 1) * 128, q * hw2 : (q + 1) * hw2],
            )
            nc.scalar.activation(
                out=wt[:], in_=wt[:], func=AF.Identity,
                bias=magic_b[:], scale=inv_sw2[:],
            )
            wq = w2qb.tile([128, hw2], BF16, tag="w2qb")
            nc.vector.tensor_scalar_add(out=wq[:], in0=wt[:], scalar1=-MAGIC)
            nc.sync.dma_start(
                out=w2q_dram[ob * 128 : (ob + 1) * 128,
                             q * hw2 : (q + 1) * hw2],
                in_=wq[:],
            )

        w2_stack = ExitStack()
        w2qTs = None
        if True:
            xqT_next = None
            for mc in range(n_chunks_a):
                if pipelined and mc == n_chunks_a - 1:
                    # last chunk: w2pools/inner1 space is free now — land the
                    # w2qT transpose-loads there so they ride the ring during
                    # this chunk's matmuls, not the transition
                    w2pools.close()
                    inner1.close()
                    w2qT_pool = w2_stack.enter_context(
                        tc.tile_pool(name="w2qT", bufs=1)
                    )
                    w2qTs = w2qT_pool.tile([128, ht, c], BF16, tag="w2qTs")
                    for jg in range(ht // 4):
                        nc.sync.dma_start_transpose(
                            out=w2qTs[:, jg * 4 : (jg + 1) * 4, :],
                            in_=w2q_dram[:, jg * 512 : (jg + 1) * 512],
                        )
                # --- sharded w2 amax scan + its AllReduce, during chunk 0 ---
                if mc == (0 if pipelined else min(1, n_chunks_a - 2)):
                    w2s_rows = c // n_cores
                    for rb in range((w2s_rows + 127) // 128):
                        nrow = min(128, w2s_rows - rb * 128)
                        wt = w2s.tile([128, h], F32, tag="w2s")
                        nc.sync.dma_start(
                            out=wt[:nrow, :],
                            in_=w2s_in[rb * 128 : rb * 128 + nrow, :],
                        )
                        wr = w2sr.tile([128, 1], F32, tag="w2sr")
                        nc.vector.tensor_reduce(
                            out=wr[:nrow], in_=wt[:nrow, :],
                            axis=mybir.AxisListType.X,
                            op=ALU.max, apply_absolute_value=True,
                        )
                        nc.vector.tensor_tensor(
                            out=wmax2[:nrow], in0=wmax2[:nrow], in1=wr[:nrow],
                            op=ALU.max
                        )
                    w2max_r = _preduce(wmax2, psH, "psH")
                    nc.gpsimd.dma_start(out=arw_in[:], in_=w2max_r[:])
                    nc.gpsimd.collective_compute(
                        "AllReduce", ALU.max, replica_groups=groups,
                        ins=[arw_in.opt()], outs=[arw_out.opt()],
                    )
                if "inv_sw2" not in sw2_state and (
                    (pipelined and mc == 0) or (not pipelined and mc in w2_quant_chunks)
                ):
                    sw2_state["sw2"], sw2_state["inv_sw2"] = _derive(
                        arw_out, "w2"
                    )
                if not pipelined and mc in w2_quant_chunks:
                    qbase = sum(v for k, v in w2_quant_chunks.items() if k < mc)
                    for k in range(w2_quant_chunks[mc]):
                        w2_quant_piece(qbase + k)

                # --- x quant for this chunk ---
                if mc == 0:
                    xqT = xqT0
                elif pipelined:
                    xqT = xqT_next
                else:
                    xqT = x_quant_chunk(mc)

                # next-chunk x prep state (pipelined emission)
                nt8 = CH // 128
                x_next = {}

                # --- matmul over j blocks; gelu; amax; store h.T fp16 ---
                for jb in range(ht):
                    if pipelined and mc + 1 < n_chunks_a:
                        # staggered load->quant->store keeps xa live <= 4
                        if jb % 2 == 0 and jb < 2 * nt8:
                            t8 = jb // 2
                            m0 = (mc + 1) * CH + t8 * 128
                            xt = xa.tile([128, c], F32, tag="xa")
                            nc.sync.dma_start(
                                out=xt[:], in_=x_in[m0 : m0 + 128, :]
                            )
                            x_next[t8] = xt
                        if jb % 2 == 1 and 5 <= jb < 5 + 2 * nt8:
                            t8 = (jb - 5) // 2
                            xt = x_next.pop(t8)
                            m0 = (mc + 1) * CH + t8 * 128
                            nc.scalar.activation(
                                out=xt[:], in_=xt[:], func=AF.Identity,
                                bias=magic_b[:], scale=inv_sx[:],
                            )
                            xq = xqp.tile([128, c], BF16, tag="xq")
                            nc.vector.tensor_scalar_add(
                                out=xq[:], in0=xt[:], scalar1=-MAGIC
                            )
                            nc.sync.dma_start(
                                out=xq_dram[m0 : m0 + 128, :], in_=xq[:]
                            )
                        elif jb == 6 + 2 * nt8:
                            xqT_next = xqtp.tile(
                                [128, ct, CH], BF16, tag="xqT",
                                name=f"xqT{mc + 1}",
                            )
                            nc.sync.dma_start_transpose(
                                out=xqT_next[:],
                                in_=xq_dram[(mc + 1) * CH : (mc + 2) * CH, :],
                            )
                    if pipelined and (mc, jb) in w2_piece_slots:
                        w2_quant_piece(w2_piece_slots[(mc, jb)])

                    phs = [
                        psH.tile([128, 512], F32, tag="psH", name=f"psH{mc}_{jb}_{i}")
                        for i in range(n_ms)
                    ]
                    prev = None
                    for cb in range(ct):
                        for ms in range(n_ms):
                            mmi = nc.tensor.matmul(
                                phs[ms][:],
                                lhsT=w1qT_ap(cb, jb),
                                rhs=xqT[:, cb, ms * 512 : (ms + 1) * 512],
                                start=(cb == 0),
                                stop=(cb == ct - 1),
                            )
                            if prev is not None:
                                _add_dep(mmi.ins, prev.ins, sync=False,
                                         reason="ldw-order")
                            prev = mmi
                    for ms in range(n_ms):
                        ph = phs[ms]
                        g = gs.tile([128, 512], F16, tag="gs")
                        if gelu == "Erf":
                            hh = gs.tile([128, 512], F32, tag="gh")
                            nc.scalar.activation(
                                out=hh[:], in_=ph[:], func=AF.Identity,
                                bias=b1_sb[:, jb : jb + 1], scale=sxw1[:],
                            )
                            e = gs.tile([128, 512], F32, tag="ge")
                            nc.scalar.activation(
                                out=e[:], in_=hh[:], func=AF.Erf, bias=0.0,
                                scale=float(1.0 / np.sqrt(2.0)),
                            )
                            nc.vector.tensor_scalar(
                                out=e[:], in0=e[:], scalar1=0.5, scalar2=0.5,
                                op0=ALU.mult, op1=ALU.add,
                            )
                            nc.vector.tensor_tensor(
                                out=g[:], in0=e[:], in1=hh[:], op=ALU.mult
                            )
                        else:
                            nc.scalar.activation(
                                out=g[:], in_=ph[:], func=getattr(AF, gelu),
                                bias=b1_sb[:, jb : jb + 1], scale=sxw1[:],
                            )
                        r = gr.tile([128, 1], F32, tag="gr")
                        nc.vector.tensor_reduce(
                            out=r[:], in_=g[:], axis=mybir.AxisListType.X,
                            op=ALU.max, apply_absolute_value=True,
                        )
                        nc.vector.tensor_tensor(
                            out=hmax[:], in0=hmax[:], in1=r[:], op=ALU.max
                        )
                        m0 = mc * CH + ms * 512
                        nc.sync.dma_start(
                            out=h_dram[jb * 128 : (jb + 1) * 128, m0 : m0 + 512],
                            in_=g[:],
                        )

        psHs.close()
        w1_stack.close()
        sw2 = sw2_state["sw2"]

        # ---------- h scale: AllReduce ----------
        with tc.tile_pool(name="psRh", bufs=2, space="PSUM") as psRh:
            hmax_r = _preduce(hmax, psRh, "hm")
        nc.gpsimd.dma_start(out=arh_in[:], in_=hmax_r[:])
        nc.gpsimd.collective_compute(
            "AllReduce", ALU.max, replica_groups=groups,
            ins=[arh_in.opt()], outs=[arh_out.opt()],
        )

        sh, inv_sh = _derive(arh_out, "h")
        shw2 = scal.tile([128, 1], F32)
        nc.vector.tensor_tensor(out=shw2[:], in0=sh[:], in1=sw2[:], op=ALU.mult)

        # ---------- phase B: y = hq.T.T @ w2q.T * (sh*sw2) + b2 ----------
        n_chunk = rows // 512
        if w2qTs is None:
            # non-pipelined fallback: materialize w2qT at the transition
            w2pools.close()
            inner1.close()
            w2qT_pool = w2_stack.enter_context(tc.tile_pool(name="w2qT", bufs=1))
            w2qTs = w2qT_pool.tile([128, ht, c], BF16, tag="w2qTs")
            for jg in range(ht // 4):
                nc.sync.dma_start_transpose(
                    out=w2qTs[:, jg * 4 : (jg + 1) * 4, :],
                    in_=w2q_dram[:, jg * 512 : (jg + 1) * 512],
                )
        with tc.tile_pool(
            name="hb", bufs=6
        ) as hb, tc.tile_pool(name="hf", bufs=3) as hf, tc.tile_pool(
            name="hqt", bufs=2
        ) as hqtp, tc.tile_pool(name="ys", bufs=4) as ys, tc.tile_pool(
            name="psY", bufs=3 * (c // 512), space="PSUM"
        ) as psY:
            def hqT_prep(mc):
                hqT = hqtp.tile([128, ht * 512], BF16, tag="hqT")
                for jb in range(ht):
                    th = hb.tile([128, 512], F16, tag="hb")
                    nc.sync.dma_start(
                        out=th[:],
                        in_=h_dram[jb * 128 : (jb + 1) * 128,
                                   mc * 512 : (mc + 1) * 512],
                    )
                    tf = hf.tile([128, 512], F32, tag="hf")
                    nc.scalar.activation(
                        out=tf[:], in_=th[:], func=AF.Identity, bias=magic_b[:],
                        scale=inv_sh[:],
                    )
                    nc.vector.tensor_scalar_add(
                        out=hqT[:, jb * 512 : (jb + 1) * 512], in0=tf[:],
                        scalar1=-MAGIC,
                    )
                return hqT

            # chunk-0 h loads: their dequant hides under the h-scale
            # AllReduce (w2qT already streamed during the last phase-A chunk)
            hqT0 = hqT_prep(0)

            for mc in range(n_chunk):
                hqT = hqT0 if mc == 0 else hqT_prep(mc)
                n_ob = c // 512
                for ms in range(4):
                    pys = [
                        psY.tile([128, 512], F32, tag="psY", name=f"psY{mc}_{ms}_{i}")
                        for i in range(n_ob)
                    ]
                    prev = None
                    for jb in range(ht):
                        for ob in range(n_ob):
                            mmi = nc.tensor.matmul(
                                pys[ob][:],
                                lhsT=hqT[:, jb * 512 + ms * 128 :
                                         jb * 512 + (ms + 1) * 128],
                                rhs=w2qTs[:, jb, ob * 512 : (ob + 1) * 512],
                                start=(jb == 0),
                                stop=(jb == ht - 1),
                            )
                            if prev is not None:
                                _add_dep(mmi.ins, prev.ins, sync=False,
                                         reason="ldw-order")
                            prev = mmi
                    for ob in range(n_ob):
                        yt = ys.tile([128, 512], F32, tag="ys")
                        nc.vector.scalar_tensor_tensor(
                            out=yt[:], in0=pys[ob][:], scalar=shw2[:],
                            in1=b2_b[:, ob * 512 : (ob + 1) * 512],
                            op0=ALU.mult, op1=ALU.add,
                        )
                        m0 = mc * 512 + ms * 128
                        nc.sync.dma_start(
                            out=y_out[m0 : m0 + 128, ob * 512 : (ob + 1) * 512],
                            in_=yt[:],
                        )

        w2_stack.close()
        pa0.close()

    if split_waits:
        _split_matmul_waits(nc)
        _dedup_ldweights(nc)
    return nc


_CACHED = {}


def _get_nc(rows, c, h, n_cores, gelu):
    key = (rows, c, h, n_cores, gelu)
    if key not in _CACHED:
        _CACHED[key] = build_nc(rows=rows, c=c, h=h, n_cores=n_cores, gelu=gelu)
    return _CACHED[key]


def run(inputs, trace=False, gelu="Gelu", n_cores=N_CORES):
    x = np.asarray(inputs["x"], np.float32)
    w1 = np.ascontiguousarray(np.asarray(inputs["w1"], np.float32))
    b1 = np.ascontiguousarray(np.asarray(inputs["b1"], np.float32))
    w2 = np.ascontiguousarray(np.asarray(inputs["w2"], np.float32))
    b2 = np.ascontiguousarray(np.asarray(inputs["b2"], np.float32))
    b_, s_, c_ = x.shape
    h_ = w1.shape[0]
    x2d = np.ascontiguousarray(x.reshape(-1, c_))
    rows = x2d.shape[0] // n_cores
    nc = _get_nc(rows, c_, h_, n_cores, gelu)
    w1r = h_ // n_cores
    w2r = c_ // n_cores
    in_maps = [
        {
            "x": np.ascontiguousarray(x2d[i * rows : (i + 1) * rows]),
            "w1": w1,
            "b1": b1,
            "w2": w2,
            "b2": b2,
            "w1s": np.ascontiguousarray(w1[i * w1r : (i + 1) * w1r]),
            "w2s": np.ascontiguousarray(w2[i * w2r : (i + 1) * w2r]),
        }
        for i in range(n_cores)
    ]
    res = run_bass_kernel_spmd(nc, in_maps, list(range(n_cores)), trace=trace)
    y2d = np.concatenate([r["y"] for r in res.results], axis=0)
    return y2d.reshape(b_, s_, c_).astype(np.float32), res


def kernel(x, w1, b1, w2, b2):
    y, _ = run({"x": x, "w1": w1, "b1": b1, "w2": w2, "b2": b2})
    return y


# revision 65
# speedup vs baseline: 1.0564x; 1.0118x over previous
"""Trainium2 Bass kernel for a fake-quantized MLP (qlinear -> gelu -> qlinear).

Reference semantics (B,S,C,H = 32,1024,1024,4096):
    x2d = x.reshape(-1, C)
    h   = round(x2d/sx) @ round(w1/sw1).T * (sx*sw1) + b1 ;  s = max(amax,eps)/127
    g   = gelu(h, exact erf)
    y   = round(g/sh) @ round(w2/sw2).T * (sh*sw2) + b2

Strategy: data-parallel over rows across 8 cores.  Quantized ints fit
exactly in bf16, so matmuls run at full bf16 rate with exact fp32 PSUM
accumulation.  Weight scales need no collective (every core scans the full
weights); the two activation scales (x, gelu output) use tiny
AllReduce(max) collectives.

v2 layout changes vs the first working version:
  - all quantized-tile transposes go through the DMA xbar (SBUF->SBUF for
    x/w1 block transposes, DRAM->SBUF bulk transpose-load for w2) instead
    of PE identity matmuls + ACT copies: keeps PE for real matmuls and
    unclogs the ACT engine that previously gated w1 prep.
  - gelu output h is staged to DRAM in fp16 (not fp32): halves the
    mid-kernel HBM traffic; fp16 rounding of g only perturbs the final
    int8 requant on ~0.1% boundary cases (noise ~1e-3 of output max).
  - w2 amax scan + quant + store of quantized-natural w2q (bf16, DRAM)
    are trickled through phase A; at the phase A->B transition w2qT is
    materialized with 8 bulk DMA transpose-loads that hide under the
    h-scale AllReduce.
  - x amax scan is emitted first so its AllReduce launches as early as
    possible; w1 scan + quant pipeline fills the AllReduce latency.
"""

import sys

import numpy as np

try:
    import concourse.bass as bass
except ImportError:  # pragma: no cover
    sys.path.insert(0, "/opt/trn_rl_repo")
    import concourse.bass as bass

import concourse.mybir as mybir
from contextlib import ExitStack
import concourse.tile as tile
from concourse.bass_utils import run_bass_kernel_spmd

from concourse.bass import _add_dep_helper as _add_dep

F32 = mybir.dt.float32
F16 = mybir.dt.float16
BF16 = mybir.dt.bfloat16
AF = mybir.ActivationFunctionType
ALU = mybir.AluOpType

QP = 127.0
EPS = 1e-6
MAGIC = 12582912.0  # 1.5 * 2**23: fp32 round-to-nearest-even integer trick

# full problem shapes
B, S, C, H = 32, 1024, 1024, 4096
N_CORES = 8


def _split_matmul_waits(nc):
    """This toolchain's walrus codegen allows only ONE sync-wait slot per
    lowered instruction (Matmult waits all land on its LDWEIGHTS since
    --enable-ldw-opt=false; queue DMAs use a single-slot DIRECT2D struct).
    Peel extra waits onto same-engine NoOps inserted just before, except for
    framework-generated drain/barrier instructions which support many."""
    n_split = 0
    for f in nc.m.functions:
        for bb in f.blocks:
            insts = bb.instructions
            out = []
            changed = False
            for inst in insts:
                si = getattr(inst, "sync_info", None)
                if si is not None and si.on_wait and len(si.on_wait) > 1:
                    waits = list(si.on_wait)
                    for k, w in enumerate(waits[:-1]):
                        nop = mybir.InstNoOp(
                            name=f"{inst.name}-wsplit{k}", ins=[], outs=[]
                        )
                        nop.engine = inst.engine
                        nop.sync_info = mybir.SyncInfo(
                            on_wait=[w], on_update=[]
                        )
                        out.append(nop)
                    inst.sync_info = mybir.SyncInfo(
                        on_wait=[waits[-1]], on_update=list(si.on_update or [])
                    )
                    n_split += 1
                    changed = True
                out.append(inst)
            if changed:
                bb.instructions = out
    return n_split


def _dedup_ldweights(nc):
    """Tile legalization emits explicit Ldweights+Matmult pairs, and walrus
    runs with --enable-ldw-opt=false, so every matmul re-streams its
    stationary operand (128 extra PE cycles on a 512-cycle matmul).  Drop an
    Ldweights whose weights AP is identical to the previous one on the PE
    stream (the PE array still holds that stationary); keep its semaphore
    effects on a NoOp."""
    n = 0
    for f in nc.m.functions:
        for bb in f.blocks:
            insts = bb.instructions
            out = []
            last_key = None
            changed = False
            for inst in insts:
                if isinstance(inst, mybir.InstLdweights):
                    key = str(inst.ins[0])
                    if key == last_key:
                        si = getattr(inst, "sync_info", None)
                        if si is not None and (si.on_wait or si.on_update):
                            nop = mybir.InstNoOp(
                                name=inst.name + "-lw", ins=[], outs=[]
                            )
                            nop.engine = inst.engine
                            nop.sync_info = si
                            out.append(nop)
                        n += 1
                        changed = True
                        continue
                    last_key = key
                elif isinstance(inst, mybir.InstMatmult):
                    if inst.is_transpose or getattr(inst, "ldweights", None):
                        last_key = None
                out.append(inst)
            if changed:
                bb.instructions = out
    return n


def build_nc(rows=4096, c=C, h=H, n_cores=N_CORES, gelu="Gelu", split_waits=True):
    """Build the per-core SPMD Bass program.

    rows: rows of x2d handled by each core.
    gelu: "Gelu" (HW ACT table), "Erf" (x*(0.5*erf(x/sqrt2)+0.5)),
          "Identity" (for simulator runs; CoreSim lacks Gelu/Erf).
    """
    assert rows % 512 == 0 and c % 512 == 0 and h % 512 == 0
    nc = bass.Bass()

    x_in = nc.dram_tensor("x", [rows, c], F32, kind="ExternalInput")
    w1_in = nc.dram_tensor("w1", [h, c], F32, kind="ExternalInput")
    b1_in = nc.dram_tensor("b1", [h], F32, kind="ExternalInput")
    w2_in = nc.dram_tensor("w2", [c, h], F32, kind="ExternalInput")
    b2_in = nc.dram_tensor("b2", [c], F32, kind="ExternalInput")
    # per-core weight shards (sliced in run()) so the amax scans are
    # 1/n_cores the traffic; the partial maxes AllReduce to the global max
    w1s_in = nc.dram_tensor("w1s", [h // n_cores, c], F32, kind="ExternalInput")
    w2s_in = nc.dram_tensor("w2s", [c // n_cores, h], F32, kind="ExternalInput")
    y_out = nc.dram_tensor("y", [rows, c], F32, kind="ExternalOutput")

    ct = c // 128   # c in 128-blocks
    ht = h // 128   # h in 128-blocks
    groups = [list(range(n_cores))]

    with tile.TileContext(nc) as tc, ExitStack() as top:
        consts = top.enter_context(tc.tile_pool(name="consts", bufs=1))
        scal = top.enter_context(tc.tile_pool(name="scal", bufs=1))
        dram = top.enter_context(tc.tile_pool(name="dram", bufs=1, space="DRAM"))

        ident_f = consts.tile([128, 128], F32)
        from concourse import masks
        masks.make_identity(nc, ident_f[:])
        ident = consts.tile([128, 128], BF16)
        masks.make_identity(nc, ident[:])

        # b1 as (128, ht): b1_sb[p, jb] = b1[jb*128 + p]
        b1_sb = consts.tile([128, ht], F32)
        nc.sync.dma_start(
            out=b1_sb[:], in_=b1_in.ap().rearrange("(a b) -> b a", b=128)
        )

        magic_b = consts.tile([128, 1], F32)
        nc.vector.memset(magic_b[:], MAGIC)

        # b2 broadcast early: its 128x4KB broadcast packets should ride the
        # idle prologue ring, not the busy phase-B transition
        b2_b = consts.tile([128, c], F32)
        nc.sync.dma_start(
            out=b2_b[:],
            in_=b2_in.ap().rearrange("(o a) -> o a", o=1).to_broadcast((128, c)),
        )

        # h scratch in DRAM, transposed + fp16: (h, rows)
        h_dram = dram.tile([h, rows], F16)
        # quantized-natural staging (bf16) for the bulk DMA transpose-loads.
        # Per-block SBUF->SBUF xbar transposes degrade into 256B packets on a
        # single HWDGE ring (measured ~25x DMA throughput collapse); bulk
        # DRAM->SBUF transpose-loads with contiguous >=1KB source rows run at
        # ~280GB/s.  x and w2 bounce through DRAM; w1 (prologue critical
        # path, PE idle anyway) is transposed on the PE instead to keep
        # 17MB off the sync ring.
        xq_dram = dram.tile([rows, c], BF16, tag="xqd")
        w2q_dram = dram.tile([c, h], BF16, tag="w2qd")
        # collective bounce buffers (DRAM, non-IO).  arx carries [xmax, w1max]
        # in one 2-element AllReduce; arw carries the sharded w2max.
        arx_in = dram.tile([2, 1], F32, tag="arxi")
        arx_out = dram.tile([2, 1], F32, tag="arxo")
        arw_in = dram.tile([1, 1], F32, tag="arwi")
        arw_out = dram.tile([1, 1], F32, tag="arwo")
        arh_in = dram.tile([1, 1], F32, tag="arhi")
        arh_out = dram.tile([1, 1], F32, tag="arho")

        def _preduce(acc, psR, tag):
            """[128,1] partition-max -> [1,1] via PE transpose + DVE reduce."""
            pt = psR.tile([1, 128], F32, tag=tag)
            nc.tensor.matmul(
                pt[:], lhsT=acc[:], rhs=ident_f[:], start=True, stop=True
            )
            out = scal.tile([1, 1], F32, name=tag + "_r")
            nc.vector.tensor_reduce(
                out=out[:], in_=pt[:], axis=mybir.AxisListType.X, op=ALU.max
            )
            return out

        def _derive(bcast_src_dram, name):
            b = scal.tile([128, 1], F32, name=name + "_b")
            nc.sync.dma_start(out=b[:], in_=bcast_src_dram.to_broadcast((128, 1)))
            s = scal.tile([128, 1], F32, name="s_" + name)
            nc.vector.tensor_scalar(
                out=s[:], in0=b[:], scalar1=EPS, scalar2=float(1.0 / QP),
                op0=ALU.max, op1=ALU.mult,
            )
            inv = scal.tile([128, 1], F32, name="inv_" + name)
            nc.vector.reciprocal(out=inv[:], in_=s[:])
            return s, inv

        # ---------- phase 0: x amax scan FIRST (its AllReduce is the long
        # pole: ~50us mesh latency).  512-row tiles, DMA-bound. ----------
        xmax = scal.tile([128, 1], F32)
        nc.vector.memset(xmax[:], 0.0)
        wmax1 = scal.tile([128, 1], F32)
        nc.vector.memset(wmax1[:], 0.0)
        wmax2 = scal.tile([128, 1], F32)
        nc.vector.memset(wmax2[:], 0.0)

        with tc.tile_pool(name="scan", bufs=4) as scanp, tc.tile_pool(
            name="scant", bufs=4
        ) as scant, tc.tile_pool(name="psR", bufs=2, space="PSUM") as psR0:
            for mb in range(rows // 512):
                t = scanp.tile([128, c * 4], F32, tag="sc")
                nc.sync.dma_start(
                    out=t[:].rearrange("b (a c) -> b a c", a=4),
                    in_=x_in[mb * 512 : (mb + 1) * 512, :].rearrange(
                        "(a b) c -> b a c", b=128
                    ),
                )
                r = scant.tile([128, 1], F32, tag="scr")
                nc.vector.tensor_reduce(
                    out=r[:], in_=t[:], axis=mybir.AxisListType.X, op=ALU.max,
                    apply_absolute_value=True,
                )
                nc.vector.tensor_tensor(
                    out=xmax[:], in0=xmax[:], in1=r[:], op=ALU.max
                )
            xmax_r = _preduce(xmax, psR0, "xm")
            nc.gpsimd.dma_start(out=arx_in[0:1, :], in_=xmax_r[:])

            # sharded w1 amax scan: this core's 1/n_cores slice only; the
            # partial max rides the same AllReduce as xmax (2 elements).
            # MUST stay on the sync HWDGE ring: the scalar ring concurrently
            # (in a NEFF that also issues xbar DMA transposes) crashes the
            # device; SWDGE/gpsimd takes ~1ms to software-generate the
            # descriptors for these strided tiles.
            w1s_rows = h // n_cores
            for rb in range((w1s_rows + 511) // 512):
                nrow = min(512, w1s_rows - rb * 512)
                nb = nrow // 128
                t = scanp.tile([128, c * nb], F32, tag="sc")
                nc.sync.dma_start(
                    out=t[:, : c * nb].rearrange("b (a c) -> b a c", a=nb),
                    in_=w1s_in[rb * 512 : rb * 512 + nrow, :].rearrange(
                        "(a b) c -> b a c", b=128
                    ),
                )
                r = scant.tile([128, 1], F32, tag="scr")
                nc.vector.tensor_reduce(
                    out=r[:], in_=t[:, : c * nb], axis=mybir.AxisListType.X,
                    op=ALU.max, apply_absolute_value=True,
                )
                nc.vector.tensor_tensor(
                    out=wmax1[:], in0=wmax1[:], in1=r[:], op=ALU.max
                )
            w1max_r = _preduce(wmax1, psR0, "w1m")
            nc.gpsimd.dma_start(out=arx_in[1:2, :], in_=w1max_r[:])
            nc.gpsimd.collective_compute(
                "AllReduce", ALU.max, replica_groups=groups,
                ins=[arx_in.opt()], outs=[arx_out.opt()],
            )

        sw1, inv_sw1 = _derive(arx_out[1:2, :], "w1")
        sx, inv_sx = _derive(arx_out[0:1, :], "x")
        sxw1 = scal.tile([128, 1], F32)
        nc.vector.tensor_tensor(out=sxw1[:], in0=sx[:], in1=sw1[:], op=ALU.mult)

        hmax = scal.tile([128, 1], F32)
        nc.vector.memset(hmax[:], 0.0)

        # ---------- w1 quant -> w1q_dram -> bulk transpose-loads ----------
        # w1qT_all layout: [128(p=c%128), jg(h/512), cb(ct), 512(j within jg)]
        # so matmul lhsT for (cb, jb) = [:, jb//4, cb, (jb%4)*128:+128].
        # Pipelined per 512-row group; emission order on the single sync ring
        # is chosen so the operands of the first matmuls land first:
        # x chunk-0 f32 loads, w1 jg=0, x chunk-0 quant+transpose, w1 jg>0.
        n_jg1 = h // 512
        w1_stack = ExitStack()
        w1qT_pool = w1_stack.enter_context(
            tc.tile_pool(name="w1qT", bufs=1, side="right")
        )
        w1qT_all = w1qT_pool.tile([128, n_jg1, ct, 512], BF16, tag="w1qT")

        def w1qT_ap(cb, jb):
            return w1qT_all[:, jb // 4, cb, (jb % 4) * 128 : (jb % 4 + 1) * 128]

        # ---------- phase A: h.T = gelu(w1q @ xq.T * (sx*sw1) + b1) -------
        CH = min(1024, rows // 2)
        n_ms = CH // 512
        n_chunks_a = rows // CH
        # w2 plan: sharded amax scan + AllReduce launched during chunk 0;
        # quant+store pieces ([128, h/2]) trickled through the last two
        # chunks (sw2 only needs to be ready by then).
        assert n_chunks_a >= 2
        hw2 = min(1024, h)
        n_w2_pieces = ct * (h // hw2)
        if n_chunks_a >= 4:
            w2_quant_chunks = {2: n_w2_pieces // 2,
                               3: n_w2_pieces - n_w2_pieces // 2}
        else:
            w2_quant_chunks = {1: n_w2_pieces}
        sw2_state = {}

        # Pool lifetime plan (SBUF pools are a LIFO stack; PSUM has its own):
        #   pa0 (gs, gr, xqt)        — open until the very end
        #   inner1 (xa, xq)          — closes after chunk-(n-2) emission
        #   w1pools (w1f, w1q)       — closes after the prologue
        #   w2pools (w2 scan/quant)  — closes after the last w2 piece
        #   w2qTp                    — opens at chunk-(n-1): its tiles land in
        #                              the space w2pools/inner1 just freed, so
        #                              the w2qT transpose-loads ride the ring
        #                              DURING the last chunk instead of
        #                              serializing into the transition.
        pa0 = ExitStack()
        gs = pa0.enter_context(tc.tile_pool(name="gs", bufs=12))
        gr = pa0.enter_context(tc.tile_pool(name="gr", bufs=8))
        xqtp = pa0.enter_context(tc.tile_pool(name="xqt", bufs=2))
        inner1 = ExitStack()
        xa = inner1.enter_context(tc.tile_pool(name="xa", bufs=5))
        xqp = inner1.enter_context(tc.tile_pool(name="xq", bufs=2))
        w1pools = ExitStack()
        w1f = w1pools.enter_context(tc.tile_pool(name="w1f", bufs=10))
        w1qp = w1pools.enter_context(tc.tile_pool(name="w1q", bufs=6))
        psT = w1pools.enter_context(tc.tile_pool(name="psT", bufs=4, space="PSUM"))

        def w1_strip_load(rb):
            wt = w1f.tile([128, c], F32, tag="w1f")
            nc.sync.dma_start(
                out=wt[:], in_=w1_in[rb * 128 : (rb + 1) * 128, :]
            )
            return wt

        def w1_strip_pipeline(rb, wt):
            """quantize -> PE-transpose 128x128 blocks into w1qT_all
            (batched 4 blocks per PSUM bank, one DVE copy each)."""
            nc.scalar.activation(
                out=wt[:], in_=wt[:], func=AF.Identity, bias=magic_b[:],
                scale=inv_sw1[:],
            )
            wq = w1qp.tile([128, c], BF16, tag="w1q")
            nc.vector.tensor_scalar_add(out=wq[:], in0=wt[:], scalar1=-MAGIC)
            jg, jsub = rb // 4, (rb % 4) * 128
            for cq in range(ct // 4):
                pt = psT.tile([128, 512], F32, tag="psT")
                for k in range(4):
                    cb = cq * 4 + k
                    nc.tensor.matmul(
                        pt[:, k * 128 : (k + 1) * 128],
                        lhsT=wq[:, cb * 128 : (cb + 1) * 128],
                        rhs=ident[:], start=True, stop=True,
                    )
                nc.vector.tensor_copy(
                    out=w1qT_all[:, jg, cq * 4 : (cq + 1) * 4,
                                 jsub : jsub + 128],
                    in_=pt[:].rearrange("p (a b) -> p a b", a=4),
                )

        def x_quant_chunk(mc, pe_transpose=False):
            xqT = xqtp.tile([128, ct, CH], BF16, tag="xqT", name=f"xqT{mc}")
            for t8 in range(CH // 128):
                m0 = mc * CH + t8 * 128
                xt = xa.tile([128, c], F32, tag="xa")
                nc.sync.dma_start(out=xt[:], in_=x_in[m0 : m0 + 128, :])
                nc.scalar.activation(
                    out=xt[:], in_=xt[:], func=AF.Identity, bias=magic_b[:],
                    scale=inv_sx[:],
                )
                xq = xqp.tile([128, c], BF16, tag="xq")
                nc.vector.tensor_scalar_add(out=xq[:], in0=xt[:], scalar1=-MAGIC)
                if pe_transpose:
                    # prologue only: PE is idle and the post-AllReduce path
                    # must not queue behind the ring
                    for cq in range(ct // 4):
                        pt = psT.tile([128, 512], F32, tag="psT")
                        for k in range(4):
                            cb = cq * 4 + k
                            nc.tensor.matmul(
                                pt[:, k * 128 : (k + 1) * 128],
                                lhsT=xq[:, cb * 128 : (cb + 1) * 128],
                                rhs=ident[:], start=True, stop=True,
                            )
                        nc.vector.tensor_copy(
                            out=xqT[:, cq * 4 : (cq + 1) * 4,
                                    t8 * 128 : (t8 + 1) * 128],
                            in_=pt[:].rearrange("p (a b) -> p a b", a=4),
                        )
                else:
                    nc.sync.dma_start(out=xq_dram[m0 : m0 + 128, :], in_=xq[:])
            if not pe_transpose:
                nc.sync.dma_start_transpose(
                    out=xqT[:],
                    in_=xq_dram[mc * CH : (mc + 1) * CH, :],
                )
            return xqT

        # Prologue interleave on the sync ring: chunk-0 x loads + quant
        # (PE-transposed, gated only by AR_x), then ALL w1 strip loads
        # contiguously (they stream through the otherwise-idle pre-AllReduce
        # ring window), then the quant+PE-transpose pipeline.
        xqT0 = x_quant_chunk(0, pe_transpose=True)
        w1_tiles = [w1_strip_load(rb) for rb in range(ht)]
        for rb in range(ht):
            w1_strip_pipeline(rb, w1_tiles[rb])
        w1pools.close()

        w2pools = ExitStack()
        w2s = w2pools.enter_context(tc.tile_pool(name="w2s", bufs=1))
        w2sr = w2pools.enter_context(tc.tile_pool(name="w2sr", bufs=2))
        w2qf = w2pools.enter_context(tc.tile_pool(name="w2qf", bufs=2))
        w2qb = w2pools.enter_context(tc.tile_pool(name="w2qb", bufs=2))
        psHs = ExitStack()
        psH = psHs.enter_context(tc.tile_pool(
            name="psH", bufs=(8 if n_ms == 2 else 3 * n_ms), space="PSUM"))

        # pipelined emission (full config): next-chunk x prep and w2-quant
        # pieces are interleaved into the jb loop so their ring traffic and
        # ACT work spread across the chunk instead of bunching at chunk
        # boundaries.
        pipelined = ht >= 32 and CH // 128 <= 8
        n_w2q = h // hw2
        w2_piece_slots = {}
        if pipelined:
            slots = []
            for mcs in range(n_chunks_a):
                for jbs in range(16 if mcs == 0 else 2, ht, 2):
                    slots.append((mcs, jbs))
            assert len(slots) >= n_w2_pieces
            for i in range(n_w2_pieces):
                w2_piece_slots[slots[i]] = i

        def w2_quant_piece(pc):
            inv_sw2 = sw2_state["inv_sw2"]
            ob, q = pc // n_w2q, pc % n_w2q
            wt = w2qf.tile([128, hw2], F32, tag="w2qf")
            nc.sync.dma_start(
                out=wt[:],
                in_=w2_in[ob * 128 : (ob + 1) * 128, q * hw2 : (q + 1) * hw2],
            )
            nc.scalar.activation(
                out=wt[:], in_=wt[:], func=AF.Identity,
                bias=magic_b[:], scale=inv_sw2[:],
            )
            wq = w2qb.tile([128, hw2], BF16, tag="w2qb")
            nc.vector.tensor_scalar_add(out=wq[:], in0=wt[:], scalar1=-MAGIC)
            nc.sync.dma_start(
                out=w2q_dram[ob * 128 : (ob + 1) * 128,
                             q * hw2 : (q + 1) * hw2],
                in_=wq[:],
            )

        w2_stack = ExitStack()
        w2qTs = None
        if True:
            xqT_next = None
            for mc in range(n_chunks_a):
                if pipelined and mc == n_chunks_a - 1:
                    # last chunk: w2pools/inner1 space is free now — land the
                    # w2qT transpose-loads there so they ride the ring during
                    # this chunk's matmuls, not the transition
                    w2pools.close()
                    inner1.close()
                    w2qT_pool = w2_stack.enter_context(
                        tc.tile_pool(name="w2qT", bufs=1)
                    )
                    w2qTs = w2qT_pool.tile([128, ht, c], BF16, tag="w2qTs")
                    for jg in range(ht // 4):
                        nc.sync.dma_start_transpose(
                            out=w2qTs[:, jg * 4 : (jg + 1) * 4, :],
                            in_=w2q_dram[:, jg * 512 : (jg + 1) * 512],
                        )
                # --- sharded w2 amax scan + its AllReduce, during chunk 0 ---
                if mc == (0 if pipelined else min(1, n_chunks_a - 2)):
                    w2s_rows = c // n_cores
                    for rb in range((w2s_rows + 127) // 128):
                        nrow = min(128, w2s_rows - rb * 128)
                        wt = w2s.tile([128, h], F32, tag="w2s")
                        nc.sync.dma_start(
                            out=wt[:nrow, :],
                            in_=w2s_in[rb * 128 : rb * 128 + nrow, :],
                        )
                        wr = w2sr.tile([128, 1], F32, tag="w2sr")
                        nc.vector.tensor_reduce(
                            out=wr[:nrow], in_=wt[:nrow, :],
                            axis=mybir.AxisListType.X,
                            op=ALU.max, apply_absolute_value=True,
                        )
                        nc.vector.tensor_tensor(
                            out=wmax2[:nrow], in0=wmax2[:nrow], in1=wr[:nrow],
                            op=ALU.max
                        )
                    w2max_r = _preduce(wmax2, psH, "psH")
                    nc.gpsimd.dma_start(out=arw_in[:], in_=w2max_r[:])
                    nc.gpsimd.collective_compute(
                        "AllReduce", ALU.max, replica_groups=groups,
                        ins=[arw_in.opt()], outs=[arw_out.opt()],
                    )
                if "inv_sw2" not in sw2_state and (
                    (pipelined and mc == 0) or (not pipelined and mc in w2_quant_chunks)
                ):
                    sw2_state["sw2"], sw2_state["inv_sw2"] = _derive(
                        arw_out, "w2"
                    )
                if not pipelined and mc in w2_quant_chunks:
                    qbase = sum(v for k, v in w2_quant_chunks.items() if k < mc)
                    for k in range(w2_quant_chunks[mc]):
                        w2_quant_piece(qbase + k)

                # --- x quant for this chunk ---
                if mc == 0:
                    xqT = xqT0
                elif pipelined:
                    xqT = xqT_next
                else:
                    xqT = x_quant_chunk(mc)

                # next-chunk x prep state (pipelined emission)
                nt8 = CH // 128
                x_next = {}

                # --- matmul over j blocks; gelu; amax; store h.T fp16 ---
                for jb in range(ht):
                    if pipelined and mc + 1 < n_chunks_a:
                        # staggered load->quant->store keeps xa live <= 4
                        if jb % 2 == 0 and jb < 2 * nt8:
                            t8 = jb // 2
                            m0 = (mc + 1) * CH + t8 * 128
                            xt = xa.tile([128, c], F32, tag="xa")
                            nc.sync.dma_start(
                                out=xt[:], in_=x_in[m0 : m0 + 128, :]
                            )
                            x_next[t8] = xt
                        if jb % 2 == 1 and 5 <= jb < 5 + 2 * nt8:
                            t8 = (jb - 5) // 2
                            xt = x_next.pop(t8)
                            m0 = (mc + 1) * CH + t8 * 128
                            nc.scalar.activation(
                                out=xt[:], in_=xt[:], func=AF.Identity,
                                bias=magic_b[:], scale=inv_sx[:],
                            )
                            xq = xqp.tile([128, c], BF16, tag="xq")
                            nc.vector.tensor_scalar_add(
                                out=xq[:], in0=xt[:], scalar1=-MAGIC
                            )
                            nc.sync.dma_start(
                                out=xq_dram[m0 : m0 + 128, :], in_=xq[:]
                            )
                        elif jb == 6 + 2 * nt8:
                            xqT_next = xqtp.tile(
                                [128, ct, CH], BF16, tag="xqT",
                                name=f"xqT{mc + 1}",
                            )
                            nc.sync.dma_start_transpose(
                                out=xqT_next[:],
                                in_=xq_dram[(mc + 1) * CH : (mc + 2) * CH, :],
                            )
                    if pipelined and (mc, jb) in w2_piece_slots:
                        w2_quant_piece(w2_piece_slots[(mc, jb)])

                    phs = [
                        psH.tile([128, 512], F32, tag="psH", name=f"psH{mc}_{jb}_{i}")
                        for i in range(n_ms)
                    ]
                    prev = None
                    for cb in range(ct):
                        for ms in range(n_ms):
                            mmi = nc.tensor.matmul(
                                phs[ms][:],
                                lhsT=w1qT_ap(cb, jb),
                                rhs=xqT[:, cb, ms * 512 : (ms + 1) * 512],
                                start=(cb == 0),
                                stop=(cb == ct - 1),
                            )
                            if prev is not None:
                                _add_dep(mmi.ins, prev.ins, sync=False,
                                         reason="ldw-order")
                            prev = mmi
                    for ms in range(n_ms):
                        ph = phs[ms]
                        g = gs.tile([128, 512], F16, tag="gs")
                        if gelu == "Erf":
                            hh = gs.tile([128, 512], F32, tag="gh")
                            nc.scalar.activation(
                                out=hh[:], in_=ph[:], func=AF.Identity,
                                bias=b1_sb[:, jb : jb + 1], scale=sxw1[:],
                            )
                            e = gs.tile([128, 512], F32, tag="ge")
                            nc.scalar.activation(
                                out=e[:], in_=hh[:], func=AF.Erf, bias=0.0,
                                scale=float(1.0 / np.sqrt(2.0)),
                            )
                            nc.vector.tensor_scalar(
                                out=e[:], in0=e[:], scalar1=0.5, scalar2=0.5,
                                op0=ALU.mult, op1=ALU.add,
                            )
                            nc.vector.tensor_tensor(
                                out=g[:], in0=e[:], in1=hh[:], op=ALU.mult
                            )
                        else:
                            nc.scalar.activation(
                                out=g[:], in_=ph[:], func=getattr(AF, gelu),
                                bias=b1_sb[:, jb : jb + 1], scale=sxw1[:],
                            )
                        r = gr.tile([128, 1], F32, tag="gr")
                        nc.vector.tensor_reduce(
                            out=r[:], in_=g[:], axis=mybir.AxisListType.X,
                            op=ALU.max, apply_absolute_value=True,
                        )
                        nc.vector.tensor_tensor(
                            out=hmax[:], in0=hmax[:], in1=r[:], op=ALU.max
                        )
                        m0 = mc * CH + ms * 512
                        nc.sync.dma_start(
                            out=h_dram[jb * 128 : (jb + 1) * 128, m0 : m0 + 512],
                            in_=g[:],
                        )

        psHs.close()
        w1_stack.close()
        sw2 = sw2_state["sw2"]

        # ---------- h scale: AllReduce ----------
        with tc.tile_pool(name="psRh", bufs=2, space="PSUM") as psRh:
            hmax_r = _preduce(hmax, psRh, "hm")
        nc.gpsimd.dma_start(out=arh_in[:], in_=hmax_r[:])
        nc.gpsimd.collective_compute(
            "AllReduce", ALU.max, replica_groups=groups,
            ins=[arh_in.opt()], outs=[arh_out.opt()],
        )

        sh, inv_sh = _derive(arh_out, "h")
        shw2 = scal.tile([128, 1], F32)
        nc.vector.tensor_tensor(out=shw2[:], in0=sh[:], in1=sw2[:], op=ALU.mult)

        # ---------- phase B: y = hq.T.T @ w2q.T * (sh*sw2) + b2 ----------
        n_chunk = rows // 512
        if w2qTs is None:
            # non-pipelined fallback: materialize w2qT at the transition
            w2pools.close()
            inner1.close()
            w2qT_pool = w2_stack.enter_context(tc.tile_pool(name="w2qT", bufs=1))
            w2qTs = w2qT_pool.tile([128, ht, c], BF16, tag="w2qTs")
            for jg in range(ht // 4):
                nc.sync.dma_start_transpose(
                    out=w2qTs[:, jg * 4 : (jg + 1) * 4, :],
                    in_=w2q_dram[:, jg * 512 : (jg + 1) * 512],
                )
        with tc.tile_pool(
            name="hb", bufs=6
        ) as hb, tc.tile_pool(name="hf", bufs=3) as hf, tc.tile_pool(
            name="hqt", bufs=2
        ) as hqtp, tc.tile_pool(name="ys", bufs=4) as ys, tc.tile_pool(
            name="psY", bufs=3 * (c // 512), space="PSUM"
        ) as psY:
            def hqT_prep(mc):
                hqT = hqtp.tile([128, ht * 512], BF16, tag="hqT")
                for jb in range(ht):
                    th = hb.tile([128, 512], F16, tag="hb")
                    nc.sync.dma_start(
                        out=th[:],
                        in_=h_dram[jb * 128 : (jb + 1) * 128,
                                   mc * 512 : (mc + 1) * 512],
                    )
                    tf = hf.tile([128, 512], F32, tag="hf")
                    nc.scalar.activation(
                        out=tf[:], in_=th[:], func=AF.Identity, bias=magic_b[:],
                        scale=inv_sh[:],
                    )
                    nc.vector.tensor_scalar_add(
                        out=hqT[:, jb * 512 : (jb + 1) * 512], in0=tf[:],
                        scalar1=-MAGIC,
                    )
                return hqT

            # chunk-0 h loads: their dequant hides under the h-scale
            # AllReduce (w2qT already streamed during the last phase-A chunk)
            hqT0 = hqT_prep(0)

            for mc in range(n_chunk):
                hqT = hqT0 if mc == 0 else hqT_prep(mc)
                n_ob = c // 512
                for ms in range(4):
                    pys = [
                        psY.tile([128, 512], F32, tag="psY", name=f"psY{mc}_{ms}_{i}")
                        for i in range(n_ob)
                    ]
                    prev = None
                    for jb in range(ht):
                        for ob in range(n_ob):
                            mmi = nc.tensor.matmul(
                                pys[ob][:],
                                lhsT=hqT[:, jb * 512 + ms * 128 :
                                         jb * 512 + (ms + 1) * 128],
                                rhs=w2qTs[:, jb, ob * 512 : (ob + 1) * 512],
                                start=(jb == 0),
                                stop=(jb == ht - 1),
                            )
                            if prev is not None:
                                _add_dep(mmi.ins, prev.ins, sync=False,
                                         reason="ldw-order")
                            prev = mmi
                    for ob in range(n_ob):
                        yt = ys.tile([128, 512], F32, tag="ys")
                        nc.vector.scalar_tensor_tensor(
                            out=yt[:], in0=pys[ob][:], scalar=shw2[:],
                            in1=b2_b[:, ob * 512 : (ob + 1) * 512],
                            op0=ALU.mult, op1=ALU.add,
                        )
                        m0 = mc * 512 + ms * 128
                        nc.sync.dma_start(
                            out=y_out[m0 : m0 + 128, ob * 512 : (ob + 1) * 512],
                            in_=yt[:],
                        )

        w2_stack.close()
        pa0.close()

    if split_waits:
        _split_matmul_waits(nc)
        _dedup_ldweights(nc)
    return nc


_CACHED = {}


def _get_nc(rows, c, h, n_cores, gelu):
    key = (rows, c, h, n_cores, gelu)
    if key not in _CACHED:
        _CACHED[key] = build_nc(rows=rows, c=c, h=h, n_cores=n_cores, gelu=gelu)
    return _CACHED[key]


def run(inputs, trace=False, gelu="Gelu", n_cores=N_CORES):
    x = np.asarray(inputs["x"], np.float32)
    w1 = np.ascontiguousarray(np.asarray(inputs["w1"], np.float32))
    b1 = np.ascontiguousarray(np.asarray(inputs["b1"], np.float32))
    w2 = np.ascontiguousarray(np.asarray(inputs["w2"], np.float32))
    b2 = np.ascontiguousarray(np.asarray(inputs["b2"], np.float32))
    b_, s_, c_ = x.shape
    h_ = w1.shape[0]
    x2d = np.ascontiguousarray(x.reshape(-1, c_))
    rows = x2d.shape[0] // n_cores
    nc = _get_nc(rows, c_, h_, n_cores, gelu)
    w1r = h_ // n_cores
    w2r = c_ // n_cores
    in_maps = [
        {
            "x": np.ascontiguousarray(x2d[i * rows : (i + 1) * rows]),
            "w1": w1,
            "b1": b1,
            "w2": w2,
            "b2": b2,
            "w1s": np.ascontiguousarray(w1[i * w1r : (i + 1) * w1r]),
            "w2s": np.ascontiguousarray(w2[i * w2r : (i + 1) * w2r]),
        }
        for i in range(n_cores)
    ]
    res = run_bass_kernel_spmd(nc, in_maps, list(range(n_cores)), trace=trace)
    y2d = np.concatenate([r["y"] for r in res.results], axis=0)
    return y2d.reshape(b_, s_, c_).astype(np.float32), res


def kernel(x, w1, b1, w2, b2):
    y, _ = run({"x": x, "w1": w1, "b1": b1, "w2": w2, "b2": b2})
    return y
